# revision 30
# baseline (speedup 1.0000x reference)
"""Multi-head causal attention (B=2, T=2048, D=1024, H=16) on 8 TRN2 NeuronCores.

Sharding: 2-way data parallel over batch x 4-way tensor parallel over heads
(4 heads per core). Each core computes q/k/v projections for its heads,
causal attention, and a partial output projection over its head-dim slice;
the host sums the 4 partials per batch and adds the bias.

Schedule (vs the ~213us v1 baseline; v2 ~188us):
  - All matmuls f32r (full PE rate at moving>=256; narrow score AND
    final AV matmuls are widened to 256 columns to dodge the 4x
    narrow-f32r penalty — the AV gap region of e2 is zeroed on gpsimd).
  - k.T is STACKED: one [2 heads x 64, keys] stationary serves both
    heads of a pair (the zero half-rows of the padded q moving operand
    select the head), halving kT SBUF/pad/copy cost.
  - wq streams as 4 x 256KB pieces at the head of the scalar queue so
    the first projection matmul starts ~10us (x chunk 0 heads the
    sync/gpsimd queues).
  - Phase-1 q/k/v projection blocks borrow the idle score-pool PSUM
    slots for a 4-deep accumulator rotation; PSUM->SBUF copies split
    ACT (idle then) / DVE.
  - Attention runs in two q-chunk-pair stages with a lag-3
    score->exp->AV software pipeline; exp covers both chunks of a pair
    in one wide ACT op; causal-mask multiplies run on GpSimd.
  - Softmax normalize per (ch, jq): DVE reciprocal on the [2,512]
    denominator-row pair, ONE K=2 broadcast matmul through the sel2
    selector stationary, one in-place DVE multiply reading PSUM.  ACT
    does nothing but the exp stream.
  - Projection q-chunks + leftover phase-1 units ride as rationed
    filler inside the attention loops, so their output DMAs overlap
    attention compute.
"""

import sys
import types
from collections import deque

import numpy as np
import orjson

import concourse.bass as bass
import concourse.mybir as mybir
import concourse.tile as tile
from concourse.bass_utils import run_bass_kernel_spmd

# ---------------------------------------------------------------- constants
B, T, D = 2, 2048, 1024
H = 16
HD = D // H  # 64
N_CORES = 8
TPG = 4  # tensor-parallel group size (heads split 4 ways)
HPC = H // TPG  # heads per core = 4
EPC = HPC * HD  # head-dim columns per core = 256
KI = 128  # contraction tile
NT = T // 128  # 16 t-tiles
NQ = T // 512  # 4 q-chunks
DK = D // 128  # 8 d-chunks
N_WARM = 2  # PE warmup matmuls (p-state ramp while the first DMAs land)

F32 = mybir.dt.float32
F32R = mybir.dt.float32r
BF16 = mybir.dt.bfloat16


# ------------------------------------------------- walrus single-wait fixup
def _split_excess_waits(bir: bytes) -> bytes:
    """This walrus build accepts at most one sync wait per instruction.
    Hoist excess on_wait entries onto EventSemaphore ops inserted just
    before the offending instruction on the same engine."""
    m = orjson.loads(bir)
    n = 0
    for fn in m["functions"]:
        for bb in fn["blocks"]:
            out = []
            for inst in bb["instructions"]:
                si = inst.get("sync_info")
                waits = (si or {}).get("on_wait") or []
                max_waits = 1
                if len(waits) > max_waits:
                    extra, keep = waits[:-max_waits], waits[-max_waits:]
                    for k in range(len(extra)):
                        out.append({
                            "debug": inst.get("debug", 0),
                            "engine": inst["engine"],
                            "ins": [], "outs": [],
                            "name": f"{inst['name']}-ws{n}-{k}",
                            "opcode": "EventSemaphore",
                            "sync_info": {"on_update": [],
                                          "on_wait": [extra[k]]},
                        })
                    si["on_wait"] = keep
                    n += 1
                out.append(inst)
            bb["instructions"] = out
    return orjson.dumps(m)


def _patch_nc(nc):
    orig = nc.to_json_bytes
    nc.to_json_bytes = lambda: _split_excess_waits(orig())
    return nc


# ------------------------------------------------------ NTFF hook (timing)
def install_ntff_hook():
    """Register the axon NTFF profile hook if the image's antenv lacks it.
    Only needed for trace=True runs (timing); harmless otherwise."""
    try:
        from antenv.axon_hooks import get_axon_ntff_profile_hook  # noqa: F401
        return
    except ImportError:
        pass
    try:
        import antenv
        from trn_agent_boot.trn_boot import _ntff_profile_via_ctypes
    except ImportError:
        return
    mod = types.ModuleType("antenv.axon_hooks")
    mod._hook = _ntff_profile_via_ctypes("/opt/axon/libaxon_pjrt.so")
    mod.set_axon_ntff_profile_hook = lambda h: setattr(mod, "_hook", h)
    mod.get_axon_ntff_profile_hook = lambda: mod._hook
    sys.modules["antenv.axon_hooks"] = mod
    antenv.axon_hooks = mod


# ----------------------------------------------------------- device program
def build_nc():
    nc = bass.Bass(target_bir_lowering=False)

    # DRAM I/O (f32r tensors hold IEEE fp32 bits; numpy sees float32)
    xT = nc.dram_tensor("xT", [D, T], F32R, kind="ExternalInput")
    wqT = nc.dram_tensor("wqT", [D, EPC], F32R, kind="ExternalInput")
    wkT = nc.dram_tensor("wkT", [D, EPC], F32R, kind="ExternalInput")
    wvT = nc.dram_tensor("wvT", [D, EPC], F32R, kind="ExternalInput")
    wpT = nc.dram_tensor("wpT", [EPC, D], F32R, kind="ExternalInput")
    mask = nc.dram_tensor("mask", [128, 128], F32R, kind="ExternalInput")
    out = nc.dram_tensor("out_part", [T, D], F32, kind="ExternalOutput")

    xTr = xT.rearrange("(ko ki) t -> ki ko t", ki=KI)
    wqTr = wqT.rearrange("(ko ki) e -> ki ko e", ki=KI)
    wkTr = wkT.rearrange("(ko ki) e -> ki ko e", ki=KI)
    wvTr = wvT.rearrange("(ko ki) e -> ki ko e", ki=KI)
    wpTr = wpT.rearrange("(ko ki) e -> ki ko e", ki=KI)

    ACT_COPY = mybir.ActivationFunctionType.Copy
    ACT_EXP = mybir.ActivationFunctionType.Exp

    with tile.TileContext(nc) as tc:
        with (
            tc.tile_pool(name="persist", bufs=1) as persist,
            tc.tile_pool(name="xstream", bufs=2) as xstream,
            tc.tile_pool(name="work", bufs=3) as work,
            tc.tile_pool(name="ps", bufs=2, space="PSUM") as ps,
            tc.tile_pool(name="ps_sc", bufs=2, space="PSUM") as ps_sc,
            tc.tile_pool(name="ps_av", bufs=2, space="PSUM") as ps_av,
            tc.tile_pool(name="outp", bufs=3) as outp,
        ):
            # ---- persistent SBUF state (wq split into per-ko-pair pieces
            # on its own DMA queue: the very first matmul only needs piece
            # 0, so it can start ~2us after the queue opens)
            wq_h = [persist.tile([KI, DK // 2, EPC], F32R, name=f"wq{i}")
                    for i in range(2)]
            wk_h = [persist.tile([KI, DK // 2, EPC], F32R, name=f"wk{i}")
                    for i in range(2)]
            wv_sb = persist.tile([KI, DK, EPC], F32R)
            wp_sb = persist.tile([KI, 2, D], F32R)
            mask_sb = persist.tile([128, 128], F32R)
            # q.T per (head, t-chunk), contraction zero-padded 64 -> 128:
            # f32r matmuls only hit the fast path with a full 128-row
            # stationary.  Head-even tiles hold data in rows 0:64 (zeros
            # below), head-odd in rows 64:128 (zeros above), so one
            # STACKED k stationary [2 heads x 64, keys] serves both heads
            # of a pair: the zero rows of the moving q operand kill the
            # other head's contribution.
            qT_sb = {(h, tch): persist.tile([KI, 512], F32R,
                                            name=f"qT_{h}_{tch}")
                     for h in range(HPC) for tch in range(NQ)}
            # k.T stacked per (head-pair, t-chunk): rows 0:64 head 2ch,
            # rows 64:128 head 2ch+1 (no zero padding needed)
            kT_sb = {(ch, tch): persist.tile([KI, 512], F32R,
                                             name=f"kT_{ch}_{tch}")
                     for ch in range(2) for tch in range(NQ)}
            # v with a ones column for the softmax denominator
            v_sb = [persist.tile([KI, HPC, HD + 1], F32R, name=f"v_{tt}")
                    for tt in range(NT)]
            attnT_sb = {(ch, jq): persist.tile([KI, 512], F32R,
                                               name=f"attnT_{ch}_{jq}")
                        for ch in range(2) for jq in range(NQ)}
            zbias = persist.tile([128, 1], F32)
            ones_f32 = persist.tile([128, HD], F32)
            # denominator-broadcast selectors: ones over partition halves
            ones_u_f = persist.tile([1, 128], F32)
            ones_l_f = persist.tile([1, 128], F32)
            ones_u = persist.tile([1, 128], F32R)
            ones_l = persist.tile([1, 128], F32R)
            zeros_f = persist.tile([HD, 512], F32)
            zf128 = persist.tile([128, 128], F32)
            warm_a = persist.tile([128, 128], BF16)
            warm_b = persist.tile([128, 512], BF16)
            act_warm = persist.tile([1, 1], F32)

            # ---- DMAs. The first q-projection matmul needs only wq
            # piece 0 (256KB, scalar queue head) and xs0[0] (sync queue
            # head), so it can start ~2.5us after the queues open.
            def phase1_dma(tch):
                # per-ko x tiles so the first accumulation matmuls start
                # as soon as their slice lands; each chunk splits across
                # two queues (per-queue DMA bandwidth is ~110-135GB/s)
                xs = [xstream.tile([KI, 512], F32R, tag=f"xs{ko}",
                                   name=f"xs_{tch}_{ko}")
                      for ko in range(DK)]
                for ko in range(DK):
                    if tch == 0:
                        xdma = nc.sync if ko < 4 else nc.gpsimd
                    else:
                        xdma = nc.sync if ko < 4 else nc.scalar
                    xdma.dma_start(
                        xs[ko][:], xTr[:, ko, tch * 512:(tch + 1) * 512])
                return xs

            # queue plans (per-queue order = arrival order):
            #   scalar: wq p0-p3 (256KB each), wk_hi, x1 ko4-7,
            #           x2 ko4-7, x3 ko4-7
            #   sync:   x0 ko0-3, wk_lo, x1 ko0-3, x2 ko0-3, x3 ko0-3
            #   gpsimd: x0 ko4-7, wv, mask, wp
            for p in range(4):
                nc.scalar.dma_start(wq_h[p // 2][:, 2 * (p % 2):
                                                 2 * (p % 2) + 2, :],
                                    wqTr[:, 2 * p:2 * p + 2, :])
            xs0 = phase1_dma(0)
            nc.sync.dma_start(wk_h[0][:], wkTr[:, 0:4, :])
            nc.scalar.dma_start(wk_h[1][:], wkTr[:, 4:8, :])
            nc.gpsimd.dma_start(wv_sb[:], wvTr[:])
            xs1 = phase1_dma(1)
            nc.gpsimd.dma_start(mask_sb[:], mask[:])
            nc.gpsimd.dma_start(wp_sb[:], wpTr[:])
            xs2 = phase1_dma(2)
            xs3 = phase1_dma(3)

            nc.vector.memset(warm_a[:], 0.0)
            nc.vector.memset(warm_b[:], 0.0)
            # PE p-state warmup: dummy matmuls (no DMA deps) keep the PE
            # busy from t~0 so the clock is ramped when real work arrives.
            for i in range(N_WARM):
                wacc = ps.tile([128, 512], F32, tag="mm", name=f"warm{i}")
                nc.tensor.matmul(wacc[:], warm_a[:], warm_b[:],
                                 start=True, stop=True)
            # ACT warmup: absorb the ~1.3us activation-table load and
            # engine cold start before the first real copy/exp
            nc.scalar.activation(act_warm[:], warm_b[0:1, 0:1],
                                 ACT_EXP, bias=0.0, scale=1.0)

            # zero the complementary q.T half-rows on DVE during the
            # initial DMA wait (chunks 2,3 fill inside phase-1 sections,
            # just ahead of first use).  Stacked k.T needs no padding.
            nc.vector.memset(zeros_f[:], 0.0)
            nc.vector.memset(zf128[:], 0.0)

            def pad_fill(tch):
                for h in range(HPC):
                    rows = slice(HD, 128) if h % 2 == 0 else slice(0, HD)
                    nc.vector.tensor_copy(qT_sb[(h, tch)][rows, :],
                                          zeros_f[:])
            pad_fill(0)
            pad_fill(1)

            nc.vector.memset(zbias[:], 0.0)
            nc.vector.memset(ones_f32[:], 1.0)
            nc.vector.memset(ones_u_f[:], 0.0)
            nc.vector.memset(ones_l_f[:], 0.0)
            nc.vector.tensor_copy(ones_u_f[0:1, 0:HD], ones_f32[0:1, :])
            nc.vector.tensor_copy(ones_l_f[0:1, HD:128], ones_f32[0:1, :])
            nc.vector.tensor_copy(ones_u[:], ones_u_f[:])
            nc.vector.tensor_copy(ones_l[:], ones_l_f[:])
            for tt in range(NT):
                nc.vector.tensor_copy(
                    v_sb[tt][:, :, HD:HD + 1].rearrange("p b c -> p (b c)"),
                    ones_f32[:, 0:HPC])

            # ---- phase 1: q.T/k.T [e,t] and v [t,e] projections per
            # 512-wide t-chunk, decomposed into filler units (one PSUM
            # accumulation group each) so chunks 2,3 can interleave into
            # the ACT-bound attention loops. PSUM->SBUF copies: q/k top
            # half on ACT when it is idle (chunks 0,1), rest on DVE.
            def qk_unit(tch, xs, qk, ec, on_act, acc_pool, acc_tag):
                w_h = wq_h if qk == 0 else wk_h
                acc = acc_pool.tile([128, 512], F32, tag=acc_tag,
                                    name=f"qk_{tch}_{qk}_{ec}")
                for ko in range(DK):
                    nc.tensor.matmul(
                        acc[:],
                        w_h[ko // 4][:, ko % 4, ec * 128:(ec + 1) * 128],
                        xs[ko][:],
                        start=(ko == 0), stop=(ko == DK - 1),
                    )
                if qk == 1:
                    # k stays stacked: one full-height copy
                    if on_act:
                        nc.scalar.activation(
                            kT_sb[(ec, tch)][:], acc[:],
                            ACT_COPY, bias=0.0, scale=1.0)
                    else:
                        nc.vector.tensor_copy(kT_sb[(ec, tch)][:], acc[:])
                    return
                # scatter q heads into their padded tiles at matching
                # row offsets (even: rows 0:64, odd: rows 64:128)
                if on_act:
                    nc.scalar.activation(
                        qT_sb[(2 * ec, tch)][0:HD, :], acc[0:HD, :],
                        ACT_COPY, bias=0.0, scale=1.0)
                else:
                    nc.vector.tensor_copy(qT_sb[(2 * ec, tch)][0:HD, :],
                                          acc[0:HD, :])
                nc.vector.tensor_copy(qT_sb[(2 * ec + 1, tch)][HD:128, :],
                                      acc[HD:128, :])

            def v_unit(tch, xs, it, acc_pool, acc_tag):
                tt = tch * 4 + it
                acc = acc_pool.tile([128, EPC], F32, tag=acc_tag,
                                    name=f"v_{tt}")
                for ko in range(DK):
                    nc.tensor.matmul(
                        acc[:],
                        xs[ko][:, it * 128:(it + 1) * 128],
                        wv_sb[:, ko, :],
                        start=(ko == 0), stop=(ko == DK - 1),
                    )
                for h in range(HPC):
                    nc.vector.tensor_copy(
                        v_sb[tt][:, h, 0:HD], acc[:, h * HD:(h + 1) * HD])

            def pace(n):
                # tiny dummy matmuls into the (phase-1-idle) av pool:
                # keep the PE busy across short DMA waits so the clock
                # p-state ramp (3us continuous -> 2.4GHz) is not reset
                for i in range(n):
                    wacc = ps_av.tile([65, 512], F32, tag="av",
                                      name=f"pace{pace.n}")
                    pace.n += 1
                    nc.tensor.matmul(wacc[:], warm_a[:, 0:65], warm_b[:],
                                     start=True, stop=True)
            pace.n = 0

            def phase1(tch, xs):
                # standalone block (DMA-arrival-bound: extra compute here
                # is free); 4-deep PSUM rotation borrowing the idle score
                # pool's slots.  Unit order matches weight-arrival order
                # (wq pieces first on scalar, wk_h1 on scalar ~21us,
                # wk_h0 on sync ~21us, wv on gpsimd ~25us) and pace
                # matmuls bridge the arrival gaps so the PE p-state ramp
                # is never reset by an idle wait.
                rot = 0
                units = [(0, 0), (0, 1), (1, 1), (1, 0)]
                for i, (qk, ec) in enumerate(units):
                    pool, tag = ((ps, "mm"), (ps_sc, "sc"))[rot % 2]
                    rot += 1
                    qk_unit(tch, xs, qk, ec, True, pool, tag)
                for it in range(4):
                    pool, tag = ((ps, "mm"), (ps_sc, "sc"))[rot % 2]
                    rot += 1
                    v_unit(tch, xs, it, pool, tag)

            def phase1_units(tch, xs):
                """Filler units: ec1 q/k of early chunks ride inside
                attention pair 0; chunk-2/3 units after."""
                for qk in range(2):
                    yield ("qk0", lambda t=tch, q=qk: qk_unit(
                        t, xs, q, 0, False, ps, "mm"))
                for it in range(4):
                    yield ("v", lambda t=tch, i=it: v_unit(
                        t, xs, i, ps, "mm"))
                for qk in range(2):
                    yield ("qk1", lambda t=tch, q=qk: qk_unit(
                        t, xs, q, 1, False, ps, "mm"))

            # ---- phase 2: causal attention for one q-chunk pair,
            # all 4 heads. kt-inner; AV lags the score/exp front by 3
            # steps so it never waits on ACT.
            #
            # Softmax normalize, batched per (ch, jq): attnT holds
            # unnormalized AV; the pair's two denominator rows live in
            # the free-dim halves of one [1,1024] SBUF tile (partition-0
            # only: engines cannot address odd partition starts).  Two
            # accumulating K=1 matmuls broadcast them into one [128,512]
            # PSUM tile, ONE ACT Reciprocal (cost is free-size only)
            # inverts it, and one in-place DVE multiply applies it.
            def normalize_one(jp, ch, jq, drows):
                dr = drows[(ch, jq)]
                bc = ps.tile([128, 512], F32, tag="mm",
                             name=f"bc_{jq}_{ch}")
                nc.tensor.matmul(bc[:], ones_u[:], dr[0:1, 0:512],
                                 start=True, stop=False)
                nc.tensor.matmul(bc[:], ones_l[:], dr[0:1, 512:1024],
                                 start=False, stop=True)
                dl = work.tile([128, 512], F32, tag="dl", bufs=1,
                               name=f"dl_{jq}_{ch}")
                nc.scalar.activation(
                    dl[:], bc[:], mybir.ActivationFunctionType.Ln,
                    bias=zbias[:], scale=1.0)
                rb = work.tile([128, 512], F32, tag="rb", bufs=2,
                               name=f"rb_{jq}_{ch}")
                nc.scalar.activation(rb[:], dl[:], ACT_EXP,
                                     bias=zbias[:], scale=-1.0)
                nc.vector.tensor_mul(attnT_sb[(ch, jq)][:],
                                     attnT_sb[(ch, jq)][:], rb[:])

            # ---- phase 3: partial output projection, one t-tile per
            # unit; ko-outer so both e-chunks reuse the attnT
            # stationary. Out DMAs alternate sync/gpsimd queues (scalar
            # stays free to issue the exp stream).
            def proj_unit(tt, alt_pool=False):
                # tail-drained units alternate into the idle score pool:
                # a 2-slot ring would stall each unit on the previous
                # one's copies, perpetually resetting the PE clock ramp
                pool, tag = (ps_sc, "sc") if alt_pool else (ps, "mm")
                o_sb = outp.tile([128, D], F32, tag="o", name=f"o_{tt}")
                accs = [pool.tile([128, 512], F32, tag=tag,
                                  name=f"p_{tt}_{ec}") for ec in range(2)]
                for ko in range(2):
                    for ec in range(2):
                        nc.tensor.matmul(
                            accs[ec][:],
                            attnT_sb[(ko, tt // 4)][:, (tt % 4) * 128:
                                                    (tt % 4 + 1) * 128],
                            wp_sb[:, ko, ec * 512:(ec + 1) * 512],
                            start=(ko == 0), stop=(ko == 1),
                        )
                for ec in range(2):
                    nc.vector.tensor_copy(
                        o_sb[:, ec * 512:(ec + 1) * 512], accs[ec][:])
                if tt >= 14:
                    # the very last tiles: 3-way split, scalar (done
                    # with exps by then) takes the final piece
                    r = out[tt * 128:(tt + 1) * 128, :]
                    e1, e2_, e3 = ((nc.sync, nc.gpsimd, nc.scalar),
                                   (nc.gpsimd, nc.sync, nc.scalar))[tt - 14]
                    e1.dma_start(r[:, 0:384], o_sb[:, 0:384])
                    e2_.dma_start(r[:, 384:768], o_sb[:, 384:768])
                    e3.dma_start(r[:, 768:], o_sb[:, 768:])
                elif tt >= 12:
                    # split each tile across two queues (per-queue DMA
                    # bandwidth is the drain limiter)
                    e1, e2_ = ((nc.sync, nc.gpsimd),
                               (nc.scalar, nc.sync))[tt - 12]
                    e1.dma_start(out[tt * 128:(tt + 1) * 128, 0:512],
                                 o_sb[:, 0:512])
                    e2_.dma_start(out[tt * 128:(tt + 1) * 128, 512:],
                                  o_sb[:, 512:])
                elif tt >= 8:
                    # mostly drains in the tail: split across two queues
                    e1, e2_ = ((nc.sync, nc.gpsimd),
                               (nc.gpsimd, nc.sync))[tt % 2]
                    e1.dma_start(out[tt * 128:(tt + 1) * 128, 0:512],
                                 o_sb[:, 0:512])
                    e2_.dma_start(out[tt * 128:(tt + 1) * 128, 512:],
                                  o_sb[:, 512:])
                else:
                    dma_eng = nc.sync if tt % 2 == 0 else nc.gpsimd
                    dma_eng.dma_start(out[tt * 128:(tt + 1) * 128, :],
                                      o_sb[:])

            def phase2(jp, fillers, stride, proj_sink=None, plan=None,
                       blockwise=False):
                """Attention for q-chunk pair jp. Between kt steps,
                drain one filler unit every `stride` steps (the loop is
                ACT-bound; fillers soak up idle PE cycles). When a jq's
                normalize completes, its projection units join
                `proj_sink` (default: this pair's own filler queue).
                With `blockwise`, the LAST head's diagonal AV blocks
                stop per 128-col block and drain (copy + block
                normalize + proj + out DMA) inline, so the final q-tiles
                leave the chip while attention is still running.
                Undrained fillers are returned to the caller."""
                pair = (2 * jp, 2 * jp + 1)
                if proj_sink is None:
                    proj_sink = fillers
                drows = {}
                nstep = 0
                cooldown = [0]

                def drain_block(jq, b, av):
                    # h==3 (ch 1) only: close out q-block b of chunk jq
                    cb = 128 * b
                    bs = slice(cb, cb + 128)
                    nc.vector.tensor_copy(
                        attnT_sb[(1, jq)][HD:128, bs], av[0:HD, bs])
                    dr = drows[(1, jq)]
                    nc.vector.tensor_copy(
                        dr[0:1, 512 + cb:512 + cb + 128],
                        av[HD:HD + 1, bs])
                    # block normalize; 256-col moving dodges the narrow
                    # f32r penalty (the extra columns hold older valid
                    # denominators and are ignored)
                    off0 = cb if b == 0 else cb - 128
                    co = cb - off0
                    bc = ps.tile([128, 256], F32, tag="mm",
                                 name=f"bcB_{jq}_{b}")
                    nc.tensor.matmul(bc[:], ones_u[:],
                                     dr[0:1, off0:off0 + 256],
                                     start=True, stop=False)
                    nc.tensor.matmul(bc[:], ones_l[:],
                                     dr[0:1, 512 + off0:512 + off0 + 256],
                                     start=False, stop=True)
                    dl = work.tile([128, 128], F32, tag="dlB", bufs=2,
                                   name=f"dlB_{jq}_{b}")
                    nc.scalar.activation(
                        dl[:], bc[:, co:co + 128],
                        mybir.ActivationFunctionType.Ln,
                        bias=zbias[:], scale=1.0)
                    rbb = work.tile([128, 128], F32, tag="rbB", bufs=2,
                                    name=f"rbB_{jq}_{b}")
                    nc.scalar.activation(rbb[:], dl[:], ACT_EXP,
                                         bias=zbias[:], scale=-1.0)
                    nc.vector.tensor_mul(attnT_sb[(1, jq)][:, bs],
                                         attnT_sb[(1, jq)][:, bs],
                                         rbb[:])
                    proj_unit(4 * jq + b)

                def emit_av(h, kt, avs, exps):
                    for jq in sorted(exps):
                        rel0 = kt - 4 * jq
                        c0 = 128 * max(rel0, 0)
                        if blockwise and h == 3 and rel0 >= 0:
                            # diagonal block gets its final contribution
                            # this step: stop it alone and drain, while
                            # the columns right of it keep accumulating
                            cb = 128 * rel0
                            nc.tensor.matmul(
                                avs[jq][:, cb:cb + 128],
                                v_sb[kt][:, h, :],
                                exps[jq][:, cb:cb + 128],
                                start=False, stop=True,
                            )
                            if cb + 128 < 512:
                                nc.tensor.matmul(
                                    avs[jq][:, cb + 128:],
                                    v_sb[kt][:, h, :],
                                    exps[jq][:, cb + 128:],
                                    start=False, stop=False,
                                )
                            drain_block(jq, rel0, avs[jq])
                            continue
                        # the 128-col final AV runs at 1/4 rate (f32r
                        # narrow); widen to 256 (the 256:384 gap of e2
                        # was zeroed on DVE alongside the mask mult)
                        c0m = min(c0, 256)
                        nc.tensor.matmul(
                            avs[jq][:, c0m:],
                            v_sb[kt][:, h, :],
                            exps[jq][:, c0m:],
                            start=(kt == 0), stop=(kt == 4 * jq + 3),
                        )
                    for jq in sorted(exps):
                        if kt == 4 * jq + 3 and not (blockwise and h == 3):
                            p0 = (h % 2) * HD
                            av = avs[jq]
                            nc.vector.tensor_copy(
                                attnT_sb[(h // 2, jq)][p0:p0 + HD, :],
                                av[0:HD, :])
                            if h % 2 == 0:
                                dr = work.tile([1, 1024], F32R, tag="dr",
                                               bufs=4, name=f"dr_{h}_{jq}")
                                drows[(h // 2, jq)] = dr
                            dr = drows[(h // 2, jq)]
                            c = (h % 2) * 512
                            nc.vector.tensor_copy(
                                dr[0:1, c:c + 512], av[HD:HD + 1, :])
                            if h % 2 == 1:
                                normalize_one(jp, h // 2, jq, drows)
                                if h == 3:
                                    # both ch normalized: projection of
                                    # this jq becomes filler work
                                    for tt in range(4 * jq, 4 * jq + 4):
                                        proj_sink.append(
                                            ("proj",
                                             lambda t=tt, **kw:
                                             proj_unit(t, **kw)))

                for h in range(HPC):
                    avs = {jq: ps_av.tile([HD + 1, 512], F32, tag="av",
                                          name=f"av_{h}_{jq}")
                           for jq in pair}
                    pipeq = []
                    for kt in range(4 * (pair[1] + 1)):
                        jqs = [jq for jq in pair if kt < 4 * (jq + 1)]
                        # both q-chunks' scores into one 2-bank psum tile
                        # so a single wide ACT exp covers them
                        s2 = ps_sc.tile([128, 2, 512], F32, tag="sc",
                                        name=f"s_{h}_{pair[0]}_{kt}")
                        e2 = work.tile([128, 2, 512], F32R, tag="exp",
                                       bufs=4, name=f"e_{h}_{pair[0]}_{kt}")
                        exps = {}
                        c0s = []
                        for i, jq in enumerate(jqs):
                            rel0 = kt - 4 * jq
                            # columns below 128*rel0 are strictly above
                            # the causal diagonal: skipped
                            c0 = 128 * max(rel0, 0)
                            c0s.append(c0)
                            # f32r matmuls with moving dim <256 run at 1/4
                            # rate: widen the score matmul (exp/AV still
                            # use the true c0)
                            c0m = min(c0, 512 - 256)
                            nc.tensor.matmul(
                                s2[:, i, c0m:],
                                kT_sb[(h // 2, kt // 4)][:, (kt % 4) * 128:
                                                         (kt % 4 + 1) * 128],
                                qT_sb[(h, jq)][:, c0m:],
                                start=True, stop=True,
                            )
                            exps[jq] = e2[:, i, :]
                        width = len(jqs) * 512 - c0s[0]
                        sflat = s2.rearrange("p a b -> p (a b)")
                        eflat = e2.rearrange("p a b -> p (a b)")
                        nc.scalar.activation(
                            eflat[:, c0s[0]:c0s[0] + width],
                            sflat[:, c0s[0]:c0s[0] + width],
                            ACT_EXP, bias=zbias[:], scale=1.0)
                        for i, jq in enumerate(jqs):
                            rel0 = kt - 4 * jq
                            if rel0 >= 0:
                                c0 = 128 * rel0
                                nc.gpsimd.tensor_mul(
                                    e2[:, i, c0:c0 + 128],
                                    e2[:, i, c0:c0 + 128],
                                    mask_sb[:])
                            if rel0 == 3:
                                # zero the never-exp'd gap so the AV
                                # matmul can widen to 256 columns
                                nc.vector.tensor_copy(e2[:, i, 256:384],
                                                      zf128[:])
                        pipeq.append((kt, exps))
                        if len(pipeq) > 3:
                            k0, e0 = pipeq.pop(0)
                            emit_av(h, k0, avs, e0)
                        nstep += 1
                        if plan is not None:
                            if fillers and nstep in plan:
                                fillers.popleft()[1]()
                        else:
                            eff = (stride if fillers and
                                   fillers[0][0] != "proj" else stride + 2)
                            if fillers and nstep % eff == 0:
                                fillers.popleft()[1]()
                    for k0, e0 in pipeq:
                        emit_av(h, k0, avs, e0)
                return fillers

            # ---- schedule: x chunks stream in up front; attention pair
            # 0 absorbs phase-1 chunk-2/3 units as PE filler, pair 1
            # absorbs the remaining ec1 q/k units and all output
            # projections of finished q-chunks; only v-units (needed by
            # pair 1's first head) drain as a block between the pairs.
            # ---- schedule. Phase-1 chunks 0,1 run as full blocks (that
            # region is DMA-arrival-bound, PE has slack anyway). The
            # attention loops are only slightly ACT-heavy (~2us headroom
            # in pair 0, ~10us in pair 1), so fillers are rationed.
            phase1(0, xs0)
            # pair 0's first heads only need chunk-1's ec0 q/k tiles;
            # everything else rides as filler on an explicit drain plan:
            # chunk-1 v first (its AVs need it from step 6), then
            # chunk-1 ec1 (pair-0 heads 2,3 read it by step 32), then
            # chunk-2/3 units as their x lands
            qk_unit(1, xs1, 0, 0, True, ps, "mm")
            qk_unit(1, xs1, 1, 0, True, ps_sc, "sc")
            pad_fill(2)
            pad_fill(3)
            u2 = list(phase1_units(2, xs2))
            u3 = list(phase1_units(3, xs3))
            by_kind = {k: [u for u in u2 + u3 if u[0] == k]
                       for k in ("qk0", "v", "qk1")}
            v2 = [u for u in by_kind["v"] if u[1].__defaults__[0] == 2]
            v3 = [u for u in by_kind["v"] if u[1].__defaults__[0] == 3]
            p1_rest = deque(
                [("v", lambda i=it: v_unit(1, xs1, i, ps, "mm"))
                 for it in range(4)]
                + [("qk1", lambda q=qk: qk_unit(1, xs1, q, 1, False,
                                                ps, "mm"))
                   for qk in range(2)]
                + by_kind["qk0"][:2] + v2 + by_kind["qk0"][2:] + v3)
            jp1_fill = deque()
            left = phase2(0, p1_rest, 5, proj_sink=jp1_fill,
                          plan={1, 2, 3, 4, 8, 12, 15, 18, 21, 23, 25,
                                27, 29, 31})
            for kind, th in left:
                th()
            # chunk-2/3 ec1 q/k units drain before any proj unit: pair
            # 1's later heads read the tiles they produce
            for item in reversed(by_kind["qk1"]):
                jp1_fill.appendleft(item)
            tail = phase2(1, jp1_fill, 4, blockwise=True)
            for i, (kind, th) in enumerate(tail):
                if kind == "proj":
                    th(alt_pool=(i % 2 == 1))
                else:
                    th()

    _patch_nc(nc)
    return nc


_NC_CACHE = None


def _get_nc():
    global _NC_CACHE
    if _NC_CACHE is None:
        _NC_CACHE = build_nc()
    return _NC_CACHE


def make_in_maps(x, w_qkv, w_proj):
    """Shard full inputs into the 8 per-core input maps."""
    scale = np.float32(HD ** -0.5)
    # [t_k, t_q]: valid where t_k <= t_q
    mask01 = np.triu(np.ones((128, 128), dtype=np.float32))
    in_maps = []
    for c in range(N_CORES):
        b, g = divmod(c, TPG)
        rows = slice(EPC * g, EPC * (g + 1))
        xt = np.ascontiguousarray(x[b].T)
        wq = np.ascontiguousarray((w_qkv[rows, :] * scale).T)
        wk = np.ascontiguousarray(w_qkv[D:][rows, :].T)
        wv = np.ascontiguousarray(w_qkv[2 * D:][rows, :].T)
        wp = np.ascontiguousarray(w_proj[:, rows].T)
        in_maps.append({
            "xT": xt, "wqT": wq, "wkT": wk, "wvT": wv, "wpT": wp,
            "mask": mask01,
        })
    return in_maps


def combine_outputs(results, b_proj):
    out = np.empty((B, T, D), dtype=np.float32)
    for b in range(B):
        acc = results[TPG * b]["out_part"].astype(np.float32).copy()
        for g in range(1, TPG):
            acc += results[TPG * b + g]["out_part"]
        out[b] = acc + b_proj[None, :]
    return out


def run(x, w_qkv, w_proj, b_proj, trace=False):
    nc = _get_nc()
    if trace:
        install_ntff_hook()
    in_maps = make_in_maps(np.asarray(x), np.asarray(w_qkv), np.asarray(w_proj))
    res = run_bass_kernel_spmd(nc, in_maps, core_ids=list(range(N_CORES)),
                               trace=trace)
    out = combine_outputs(res.results, np.asarray(b_proj))
    return out, res


def kernel(x, w_qkv, w_proj, b_proj):
    out, _ = run(x, w_qkv, w_proj, b_proj, trace=False)
    return out



# revision 43
# speedup vs baseline: 1.1050x; 1.1050x over previous
"""Multi-head causal attention (B=2, T=2048, D=1024, H=16) on 8 TRN2 NeuronCores.

Sharding: 2-way data parallel over batch x 4-way tensor parallel over heads
(4 heads per core). Each core computes q/k/v projections for its heads,
causal attention, and a partial output projection over its head-dim slice;
the host sums the 4 partials per batch and adds the bias.

Schedule (vs the ~213us v1 baseline; v2 ~188us):
  - All matmuls f32r (full PE rate at moving>=256; narrow score AND
    final AV matmuls are widened to 256 columns to dodge the 4x
    narrow-f32r penalty — the AV gap region of e2 is zeroed on gpsimd).
  - k.T is STACKED: one [2 heads x 64, keys] stationary serves both
    heads of a pair (the zero half-rows of the padded q moving operand
    select the head), halving kT SBUF/pad/copy cost.
  - wq streams as 4 x 256KB pieces at the head of the scalar queue so
    the first projection matmul starts ~10us (x chunk 0 heads the
    sync/gpsimd queues).
  - Phase-1 q/k/v projection blocks borrow the idle score-pool PSUM
    slots for a 4-deep accumulator rotation; PSUM->SBUF copies split
    ACT (idle then) / DVE.
  - Attention runs in two q-chunk-pair stages with a lag-3
    score->exp->AV software pipeline; exp covers both chunks of a pair
    in one wide ACT op; causal-mask multiplies run on GpSimd.
  - Softmax normalize per (ch, jq): DVE reciprocal on the [2,512]
    denominator-row pair, ONE K=2 broadcast matmul through the sel2
    selector stationary, one in-place DVE multiply reading PSUM.  ACT
    does nothing but the exp stream.
  - Projection q-chunks + leftover phase-1 units ride as rationed
    filler inside the attention loops, so their output DMAs overlap
    attention compute.
"""

import sys
import types
from collections import deque

import numpy as np
import orjson

import concourse.bass as bass
import concourse.mybir as mybir
import concourse.tile as tile
from concourse.bass_utils import run_bass_kernel_spmd

# ---------------------------------------------------------------- constants
B, T, D = 2, 2048, 1024
H = 16
HD = D // H  # 64
N_CORES = 8
TPG = 4  # tensor-parallel group size (heads split 4 ways)
HPC = H // TPG  # heads per core = 4
EPC = HPC * HD  # head-dim columns per core = 256
KI = 128  # contraction tile
NT = T // 128  # 16 t-tiles
NQ = T // 512  # 4 q-chunks
DK = D // 128  # 8 d-chunks
N_WARM = 2  # PE warmup matmuls (p-state ramp while the first DMAs land)

F32 = mybir.dt.float32
F32R = mybir.dt.float32r
BF16 = mybir.dt.bfloat16


# ------------------------------------------------- walrus single-wait fixup
def _split_excess_waits(bir: bytes) -> bytes:
    """This walrus build accepts at most one sync wait per instruction.
    Hoist excess on_wait entries onto EventSemaphore ops inserted just
    before the offending instruction on the same engine."""
    m = orjson.loads(bir)
    n = 0
    for fn in m["functions"]:
        for bb in fn["blocks"]:
            out = []
            for inst in bb["instructions"]:
                si = inst.get("sync_info")
                waits = (si or {}).get("on_wait") or []
                max_waits = 1
                if len(waits) > max_waits:
                    extra, keep = waits[:-max_waits], waits[-max_waits:]
                    for k in range(len(extra)):
                        out.append({
                            "debug": inst.get("debug", 0),
                            "engine": inst["engine"],
                            "ins": [], "outs": [],
                            "name": f"{inst['name']}-ws{n}-{k}",
                            "opcode": "EventSemaphore",
                            "sync_info": {"on_update": [],
                                          "on_wait": [extra[k]]},
                        })
                    si["on_wait"] = keep
                    n += 1
                out.append(inst)
            bb["instructions"] = out
    return orjson.dumps(m)


def _patch_nc(nc):
    orig = nc.to_json_bytes
    nc.to_json_bytes = lambda: _split_excess_waits(orig())
    return nc


# ------------------------------------------------------ NTFF hook (timing)
def install_ntff_hook():
    """Register the axon NTFF profile hook if the image's antenv lacks it.
    Only needed for trace=True runs (timing); harmless otherwise."""
    try:
        from antenv.axon_hooks import get_axon_ntff_profile_hook  # noqa: F401
        return
    except ImportError:
        pass
    try:
        import antenv
        from trn_agent_boot.trn_boot import _ntff_profile_via_ctypes
    except ImportError:
        return
    mod = types.ModuleType("antenv.axon_hooks")
    mod._hook = _ntff_profile_via_ctypes("/opt/axon/libaxon_pjrt.so")
    mod.set_axon_ntff_profile_hook = lambda h: setattr(mod, "_hook", h)
    mod.get_axon_ntff_profile_hook = lambda: mod._hook
    sys.modules["antenv.axon_hooks"] = mod
    antenv.axon_hooks = mod


# ----------------------------------------------------------- device program
def build_nc():
    nc = bass.Bass(target_bir_lowering=False)

    # DRAM I/O (f32r tensors hold IEEE fp32 bits; numpy sees float32)
    xT = nc.dram_tensor("xT", [D, T], F32R, kind="ExternalInput")
    wqT = nc.dram_tensor("wqT", [D, EPC], F32R, kind="ExternalInput")
    wkT = nc.dram_tensor("wkT", [D, EPC], F32R, kind="ExternalInput")
    wvT = nc.dram_tensor("wvT", [D, EPC], F32R, kind="ExternalInput")
    wpT = nc.dram_tensor("wpT", [EPC, D], F32R, kind="ExternalInput")
    mask = nc.dram_tensor("mask", [128, 128], F32R, kind="ExternalInput")
    out = nc.dram_tensor("out_part", [T, D], F32, kind="ExternalOutput")

    xTr = xT.rearrange("(ko ki) t -> ki ko t", ki=KI)
    wqTr = wqT.rearrange("(ko ki) e -> ki ko e", ki=KI)
    wkTr = wkT.rearrange("(ko ki) e -> ki ko e", ki=KI)
    wvTr = wvT.rearrange("(ko ki) e -> ki ko e", ki=KI)
    wpTr = wpT.rearrange("(ko ki) e -> ki ko e", ki=KI)

    ACT_COPY = mybir.ActivationFunctionType.Copy
    ACT_EXP = mybir.ActivationFunctionType.Exp

    with tile.TileContext(nc) as tc:
        with (
            tc.tile_pool(name="persist", bufs=1) as persist,
            tc.tile_pool(name="xstream", bufs=2) as xstream,
            tc.tile_pool(name="work", bufs=3) as work,
            tc.tile_pool(name="ps", bufs=2, space="PSUM") as ps,
            tc.tile_pool(name="ps_sc", bufs=2, space="PSUM") as ps_sc,
            tc.tile_pool(name="ps_av", bufs=2, space="PSUM") as ps_av,
            tc.tile_pool(name="outp", bufs=3) as outp,
        ):
            # ---- persistent SBUF state (wq split into per-ko-pair pieces
            # on its own DMA queue: the very first matmul only needs piece
            # 0, so it can start ~2us after the queue opens)
            wq_h = [persist.tile([KI, DK // 2, EPC], F32R, name=f"wq{i}")
                    for i in range(2)]
            wk_h = [persist.tile([KI, DK // 2, EPC], F32R, name=f"wk{i}")
                    for i in range(2)]
            wv_sb = persist.tile([KI, DK, EPC], F32R)
            wp_sb = persist.tile([KI, 2, D], F32R)
            mask_sb = persist.tile([128, 128], F32R)
            # q.T per (head, t-chunk), contraction zero-padded 64 -> 128:
            # f32r matmuls only hit the fast path with a full 128-row
            # stationary.  Head-even tiles hold data in rows 0:64 (zeros
            # below), head-odd in rows 64:128 (zeros above), so one
            # STACKED k stationary [2 heads x 64, keys] serves both heads
            # of a pair: the zero rows of the moving q operand kill the
            # other head's contribution.
            qT_sb = {(h, tch): persist.tile([KI, 512], F32R,
                                            name=f"qT_{h}_{tch}")
                     for h in range(HPC) for tch in range(NQ)}
            # k.T stacked per (head-pair, t-chunk): rows 0:64 head 2ch,
            # rows 64:128 head 2ch+1 (no zero padding needed)
            kT_sb = {(ch, tch): persist.tile([KI, 512], F32R,
                                             name=f"kT_{ch}_{tch}")
                     for ch in range(2) for tch in range(NQ)}
            # v with a ones column for the softmax denominator
            v_sb = [persist.tile([KI, HPC, HD + 1], F32R, name=f"v_{tt}")
                    for tt in range(NT)]
            attnT_sb = {(ch, jq): persist.tile([KI, 512], F32R,
                                               name=f"attnT_{ch}_{jq}")
                        for ch in range(2) for jq in range(NQ)}
            zbias = persist.tile([128, 1], F32)
            ones_f32 = persist.tile([128, HD], F32)
            # denominator-broadcast selectors: ones over partition halves
            ones_u_f = persist.tile([1, 128], F32)
            ones_l_f = persist.tile([1, 128], F32)
            ones_u = persist.tile([1, 128], F32R)
            ones_l = persist.tile([1, 128], F32R)
            zeros_f = persist.tile([HD, 512], F32)
            zf128 = persist.tile([128, 128], F32)
            warm_a = persist.tile([128, 128], BF16)
            warm_b = persist.tile([128, 512], BF16)
            act_warm = persist.tile([1, 1], F32)

            # ---- DMAs. The first q-projection matmul needs only wq
            # piece 0 (256KB, scalar queue head) and xs0[0] (sync queue
            # head), so it can start ~2.5us after the queues open.
            def phase1_dma(tch):
                # per-ko x tiles so the first accumulation matmuls start
                # as soon as their slice lands; each chunk splits across
                # two queues (per-queue DMA bandwidth is ~110-135GB/s)
                xs = [xstream.tile([KI, 512], F32R, tag=f"xs{ko}",
                                   name=f"xs_{tch}_{ko}")
                      for ko in range(DK)]
                for ko in range(DK):
                    # interleave the two queues in ko (consumption)
                    # order: the accumulation matmuls eat tiles in ko
                    # order, so alternating queues halves the effective
                    # arrival cadence
                    if tch == 0:
                        xdma = nc.sync if ko % 2 == 0 else nc.gpsimd
                    else:
                        xdma = nc.sync if ko % 2 == 0 else nc.scalar
                    xdma.dma_start(
                        xs[ko][:], xTr[:, ko, tch * 512:(tch + 1) * 512])
                return xs

            # queue plans (per-queue order = arrival order):
            #   scalar: wq p0-p3 (256KB each), wk_hi, x1 ko4-7,
            #           x2 ko4-7, x3 ko4-7
            #   sync:   x0 ko0-3, wk_lo, x1 ko0-3, x2 ko0-3, x3 ko0-3
            #   gpsimd: x0 ko4-7, wv, mask, wp
            for p in range(4):
                nc.scalar.dma_start(wq_h[p // 2][:, 2 * (p % 2):
                                                 2 * (p % 2) + 2, :],
                                    wqTr[:, 2 * p:2 * p + 2, :])
            xs0 = phase1_dma(0)
            nc.sync.dma_start(wk_h[0][:], wkTr[:, 0:4, :])
            nc.scalar.dma_start(wk_h[1][:], wkTr[:, 4:8, :])
            nc.gpsimd.dma_start(wv_sb[:], wvTr[:])
            xs1 = phase1_dma(1)
            nc.gpsimd.dma_start(mask_sb[:], mask[:])
            nc.gpsimd.dma_start(wp_sb[:], wpTr[:])
            xs2 = phase1_dma(2)
            xs3 = phase1_dma(3)

            nc.vector.memset(warm_a[:], 0.0)
            nc.vector.memset(warm_b[:], 0.0)
            # PE p-state warmup: dummy matmuls (no DMA deps) keep the PE
            # busy from t~0 so the clock is ramped when real work arrives.
            for i in range(N_WARM):
                wacc = ps.tile([128, 512], F32, tag="mm", name=f"warm{i}")
                nc.tensor.matmul(wacc[:], warm_a[:], warm_b[:],
                                 start=True, stop=True)
            # ACT warmup: absorb the ~1.3us activation-table load and
            # engine cold start before the first real copy/exp
            nc.scalar.activation(act_warm[:], warm_b[0:1, 0:1],
                                 ACT_EXP, bias=0.0, scale=1.0)

            # zero the complementary q.T half-rows on DVE during the
            # initial DMA wait (chunks 2,3 fill inside phase-1 sections,
            # just ahead of first use).  Stacked k.T needs no padding.
            nc.vector.memset(zeros_f[:], 0.0)
            nc.vector.memset(zf128[:], 0.0)

            def pad_fill(tch):
                for h in range(HPC):
                    rows = slice(HD, 128) if h % 2 == 0 else slice(0, HD)
                    nc.vector.tensor_copy(qT_sb[(h, tch)][rows, :],
                                          zeros_f[:])
            pad_fill(0)
            pad_fill(1)

            nc.vector.memset(zbias[:], 0.0)
            nc.vector.memset(ones_f32[:], 1.0)
            nc.vector.memset(ones_u_f[:], 0.0)
            nc.vector.memset(ones_l_f[:], 0.0)
            nc.vector.tensor_copy(ones_u_f[0:1, 0:HD], ones_f32[0:1, :])
            nc.vector.tensor_copy(ones_l_f[0:1, HD:128], ones_f32[0:1, :])
            nc.vector.tensor_copy(ones_u[:], ones_u_f[:])
            nc.vector.tensor_copy(ones_l[:], ones_l_f[:])
            for tt in range(NT):
                nc.vector.tensor_copy(
                    v_sb[tt][:, :, HD:HD + 1].rearrange("p b c -> p (b c)"),
                    ones_f32[:, 0:HPC])

            # ---- phase 1: q.T/k.T [e,t] and v [t,e] projections per
            # 512-wide t-chunk, decomposed into filler units (one PSUM
            # accumulation group each) so chunks 2,3 can interleave into
            # the ACT-bound attention loops. PSUM->SBUF copies: q/k top
            # half on ACT when it is idle (chunks 0,1), rest on DVE.
            def qk_unit(tch, xs, qk, ec, on_act, acc_pool, acc_tag):
                w_h = wq_h if qk == 0 else wk_h
                acc = acc_pool.tile([128, 512], F32, tag=acc_tag,
                                    name=f"qk_{tch}_{qk}_{ec}")
                for ko in range(DK):
                    nc.tensor.matmul(
                        acc[:],
                        w_h[ko // 4][:, ko % 4, ec * 128:(ec + 1) * 128],
                        xs[ko][:],
                        start=(ko == 0), stop=(ko == DK - 1),
                    )
                if qk == 1:
                    # k stays stacked: one full-height copy
                    if on_act:
                        nc.scalar.activation(
                            kT_sb[(ec, tch)][:], acc[:],
                            ACT_COPY, bias=0.0, scale=1.0)
                    else:
                        nc.vector.tensor_copy(kT_sb[(ec, tch)][:], acc[:])
                    return
                # scatter q heads into their padded tiles at matching
                # row offsets (even: rows 0:64, odd: rows 64:128)
                if on_act:
                    nc.scalar.activation(
                        qT_sb[(2 * ec, tch)][0:HD, :], acc[0:HD, :],
                        ACT_COPY, bias=0.0, scale=1.0)
                else:
                    nc.vector.tensor_copy(qT_sb[(2 * ec, tch)][0:HD, :],
                                          acc[0:HD, :])
                nc.vector.tensor_copy(qT_sb[(2 * ec + 1, tch)][HD:128, :],
                                      acc[HD:128, :])

            def v_unit(tch, xs, it, acc_pool, acc_tag):
                tt = tch * 4 + it
                acc = acc_pool.tile([128, EPC], F32, tag=acc_tag,
                                    name=f"v_{tt}")
                for ko in range(DK):
                    nc.tensor.matmul(
                        acc[:],
                        xs[ko][:, it * 128:(it + 1) * 128],
                        wv_sb[:, ko, :],
                        start=(ko == 0), stop=(ko == DK - 1),
                    )
                for h in range(HPC):
                    nc.vector.tensor_copy(
                        v_sb[tt][:, h, 0:HD], acc[:, h * HD:(h + 1) * HD])

            def pace(n):
                # tiny dummy matmuls into the (phase-1-idle) av pool:
                # keep the PE busy across short DMA waits so the clock
                # p-state ramp (3us continuous -> 2.4GHz) is not reset
                for i in range(n):
                    wacc = ps_av.tile([65, 512], F32, tag="av",
                                      name=f"pace{pace.n}")
                    pace.n += 1
                    nc.tensor.matmul(wacc[:], warm_a[:, 0:65], warm_b[:],
                                     start=True, stop=True)
            pace.n = 0

            def phase1(tch, xs):
                # standalone block (DMA-arrival-bound: extra compute here
                # is free); 4-deep PSUM rotation borrowing the idle score
                # pool's slots.  Unit order matches weight-arrival order
                # (wq pieces first on scalar, wk_h1 on scalar ~21us,
                # wk_h0 on sync ~21us, wv on gpsimd ~25us) and pace
                # matmuls bridge the arrival gaps so the PE p-state ramp
                # is never reset by an idle wait.
                rot = 0
                units = [(0, 0), (0, 1), (1, 1), (1, 0)]
                for i, (qk, ec) in enumerate(units):
                    pool, tag = ((ps, "mm"), (ps_sc, "sc"))[rot % 2]
                    rot += 1
                    qk_unit(tch, xs, qk, ec, True, pool, tag)
                for it in range(4):
                    pool, tag = ((ps, "mm"), (ps_sc, "sc"))[rot % 2]
                    rot += 1
                    v_unit(tch, xs, it, pool, tag)

            def phase1_units(tch, xs):
                """Filler units: ec1 q/k of early chunks ride inside
                attention pair 0; chunk-2/3 units after."""
                for qk in range(2):
                    yield ("qk0", lambda t=tch, q=qk: qk_unit(
                        t, xs, q, 0, False, ps, "mm"))
                for it in range(4):
                    yield ("v", lambda t=tch, i=it: v_unit(
                        t, xs, i, ps, "mm"))
                for qk in range(2):
                    yield ("qk1", lambda t=tch, q=qk: qk_unit(
                        t, xs, q, 1, False, ps, "mm"))

            # ---- phase 2: causal attention for one q-chunk pair,
            # all 4 heads. kt-inner; AV lags the score/exp front by 3
            # steps so it never waits on ACT.
            #
            # Softmax normalize, batched per (ch, jq): attnT holds
            # unnormalized AV; the pair's two denominator rows live in
            # the free-dim halves of one [1,1024] SBUF tile (partition-0
            # only: engines cannot address odd partition starts).  Two
            # accumulating K=1 matmuls broadcast them into one [128,512]
            # PSUM tile, ONE ACT Reciprocal (cost is free-size only)
            # inverts it, and one in-place DVE multiply applies it.
            def normalize_one(jp, ch, jq, drows):
                dr = drows[(ch, jq)]
                bc = ps.tile([128, 512], F32, tag="mm",
                             name=f"bc_{jq}_{ch}")
                nc.tensor.matmul(bc[:], ones_u[:], dr[0:1, 0:512],
                                 start=True, stop=False)
                nc.tensor.matmul(bc[:], ones_l[:], dr[0:1, 512:1024],
                                 start=False, stop=True)
                dl = work.tile([128, 512], F32, tag="dl", bufs=1,
                               name=f"dl_{jq}_{ch}")
                nc.scalar.activation(
                    dl[:], bc[:], mybir.ActivationFunctionType.Ln,
                    bias=zbias[:], scale=1.0)
                rb = work.tile([128, 512], F32, tag="rb", bufs=2,
                               name=f"rb_{jq}_{ch}")
                nc.scalar.activation(rb[:], dl[:], ACT_EXP,
                                     bias=zbias[:], scale=-1.0)
                nc.vector.tensor_mul(attnT_sb[(ch, jq)][:],
                                     attnT_sb[(ch, jq)][:], rb[:])

            # ---- phase 3: partial output projection, one t-tile per
            # unit; ko-outer so both e-chunks reuse the attnT
            # stationary. Out DMAs alternate sync/gpsimd queues (scalar
            # stays free to issue the exp stream).
            def proj_unit(tt, alt_pool=False):
                # tail-drained units alternate into the idle score pool:
                # a 2-slot ring would stall each unit on the previous
                # one's copies, perpetually resetting the PE clock ramp
                pool, tag = (ps_sc, "sc") if alt_pool else (ps, "mm")
                o_sb = outp.tile([128, D], F32, tag="o", name=f"o_{tt}")
                accs = [pool.tile([128, 512], F32, tag=tag,
                                  name=f"p_{tt}_{ec}") for ec in range(2)]
                for ko in range(2):
                    for ec in range(2):
                        nc.tensor.matmul(
                            accs[ec][:],
                            attnT_sb[(ko, tt // 4)][:, (tt % 4) * 128:
                                                    (tt % 4 + 1) * 128],
                            wp_sb[:, ko, ec * 512:(ec + 1) * 512],
                            start=(ko == 0), stop=(ko == 1),
                        )
                r = out[tt * 128:(tt + 1) * 128, :]
                for ec in range(2):
                    # copy then IMMEDIATELY drain that half: the ec0
                    # half's DMA overlaps the ec1 copy
                    nc.vector.tensor_copy(
                        o_sb[:, ec * 512:(ec + 1) * 512], accs[ec][:])
                    if tt >= 12:
                        # jq3 tiles drain after the last exp: 2 queue
                        # pieces per half, scalar joins in
                        qs = ((nc.sync, nc.gpsimd, nc.scalar, nc.sync),
                              (nc.gpsimd, nc.scalar, nc.sync, nc.gpsimd),
                              (nc.scalar, nc.sync, nc.gpsimd, nc.scalar),
                              (nc.sync, nc.gpsimd, nc.scalar, nc.gpsimd),
                              )[tt - 12]
                        for p in range(2):
                            c0_ = ec * 512 + p * 256
                            qs[2 * ec + p].dma_start(
                                r[:, c0_:c0_ + 256],
                                o_sb[:, c0_:c0_ + 256])
                    elif tt >= 8:
                        e1, e2_ = ((nc.sync, nc.gpsimd),
                                   (nc.gpsimd, nc.sync))[tt % 2]
                        (e1 if ec == 0 else e2_).dma_start(
                            r[:, ec * 512:(ec + 1) * 512],
                            o_sb[:, ec * 512:(ec + 1) * 512])
                    elif ec == 1:
                        dma_eng = nc.sync if tt % 2 == 0 else nc.gpsimd
                        dma_eng.dma_start(r[:], o_sb[:])

            def phase2(jp, fillers, stride, proj_sink=None, plan=None):
                """Attention for q-chunk pair jp. Between kt steps,
                drain one filler unit every `stride` steps (the loop is
                ACT-bound; fillers soak up idle PE cycles). When a jq's
                normalize completes, its projection units join
                `proj_sink` (default: this pair's own filler queue).
                Undrained fillers are returned to the caller."""
                pair = (2 * jp, 2 * jp + 1)
                if proj_sink is None:
                    proj_sink = fillers
                drows = {}
                nstep = 0
                cooldown = [0]

                def emit_av(h, kt, avs, exps):
                    for jq in sorted(exps):
                        rel0 = kt - 4 * jq
                        c0 = 128 * max(rel0, 0)
                        # the 128-col final AV runs at 1/4 rate (f32r
                        # narrow); widen to 256 (the 256:384 gap of e2
                        # was zeroed on DVE alongside the mask mult)
                        c0m = min(c0, 256)
                        nc.tensor.matmul(
                            avs[jq][:, c0m:],
                            v_sb[kt][:, h, :],
                            exps[jq][:, c0m:],
                            start=(kt == 0), stop=(kt == 4 * jq + 3),
                        )
                    for jq in sorted(exps):
                        if kt == 4 * jq + 3:
                            p0 = (h % 2) * HD
                            av = avs[jq]
                            nc.vector.tensor_copy(
                                attnT_sb[(h // 2, jq)][p0:p0 + HD, :],
                                av[0:HD, :])
                            if h % 2 == 0:
                                dr = work.tile([1, 1024], F32R, tag="dr",
                                               bufs=4, name=f"dr_{h}_{jq}")
                                drows[(h // 2, jq)] = dr
                            dr = drows[(h // 2, jq)]
                            c = (h % 2) * 512
                            nc.vector.tensor_copy(
                                dr[0:1, c:c + 512], av[HD:HD + 1, :])
                            if h % 2 == 1:
                                normalize_one(jp, h // 2, jq, drows)
                                if h == 3:
                                    # both ch normalized: projection of
                                    # this jq becomes filler work
                                    for tt in range(4 * jq, 4 * jq + 4):
                                        proj_sink.append(
                                            ("proj",
                                             lambda t=tt, **kw:
                                             proj_unit(t, **kw)))

                for h in range(HPC):
                    avs = {jq: ps_av.tile([HD + 1, 512], F32, tag="av",
                                          name=f"av_{h}_{jq}")
                           for jq in pair}
                    pipeq = []
                    for kt in range(4 * (pair[1] + 1)):
                        jqs = [jq for jq in pair if kt < 4 * (jq + 1)]
                        # both q-chunks' scores into one 2-bank psum tile
                        # so a single wide ACT exp covers them
                        s2 = ps_sc.tile([128, 2, 512], F32, tag="sc",
                                        name=f"s_{h}_{pair[0]}_{kt}")
                        e2 = work.tile([128, 2, 512], F32R, tag="exp",
                                       bufs=4, name=f"e_{h}_{pair[0]}_{kt}")
                        exps = {}
                        c0s = []
                        for i, jq in enumerate(jqs):
                            rel0 = kt - 4 * jq
                            # columns below 128*rel0 are strictly above
                            # the causal diagonal: skipped
                            c0 = 128 * max(rel0, 0)
                            c0s.append(c0)
                            # f32r matmuls with moving dim <256 run at 1/4
                            # rate: widen the score matmul (exp/AV still
                            # use the true c0)
                            c0m = min(c0, 512 - 256)
                            nc.tensor.matmul(
                                s2[:, i, c0m:],
                                kT_sb[(h // 2, kt // 4)][:, (kt % 4) * 128:
                                                         (kt % 4 + 1) * 128],
                                qT_sb[(h, jq)][:, c0m:],
                                start=True, stop=True,
                            )
                            exps[jq] = e2[:, i, :]
                        width = len(jqs) * 512 - c0s[0]
                        sflat = s2.rearrange("p a b -> p (a b)")
                        eflat = e2.rearrange("p a b -> p (a b)")
                        nc.scalar.activation(
                            eflat[:, c0s[0]:c0s[0] + width],
                            sflat[:, c0s[0]:c0s[0] + width],
                            ACT_EXP, bias=zbias[:], scale=1.0)
                        for i, jq in enumerate(jqs):
                            rel0 = kt - 4 * jq
                            if rel0 >= 0:
                                c0 = 128 * rel0
                                nc.gpsimd.tensor_mul(
                                    e2[:, i, c0:c0 + 128],
                                    e2[:, i, c0:c0 + 128],
                                    mask_sb[:])
                            if rel0 == 3:
                                # zero the never-exp'd gap so the AV
                                # matmul can widen to 256 columns
                                nc.vector.tensor_copy(e2[:, i, 256:384],
                                                      zf128[:])
                        pipeq.append((kt, exps))
                        if len(pipeq) > 3:
                            k0, e0 = pipeq.pop(0)
                            emit_av(h, k0, avs, e0)
                        nstep += 1
                        if plan is not None:
                            if fillers and nstep in plan:
                                fillers.popleft()[1]()
                        else:
                            eff = (stride if fillers and
                                   fillers[0][0] != "proj" else stride + 1)
                            if fillers and nstep % eff == 0:
                                fillers.popleft()[1]()
                    for k0, e0 in pipeq:
                        emit_av(h, k0, avs, e0)
                return fillers

            # ---- schedule: x chunks stream in up front; attention pair
            # 0 absorbs phase-1 chunk-2/3 units as PE filler, pair 1
            # absorbs the remaining ec1 q/k units and all output
            # projections of finished q-chunks; only v-units (needed by
            # pair 1's first head) drain as a block between the pairs.
            # ---- schedule. Phase-1 chunks 0,1 run as full blocks (that
            # region is DMA-arrival-bound, PE has slack anyway). The
            # attention loops are only slightly ACT-heavy (~2us headroom
            # in pair 0, ~10us in pair 1), so fillers are rationed.
            phase1(0, xs0)
            # pair 0's first heads only need chunk-1's ec0 q/k tiles;
            # everything else rides as filler on an explicit drain plan:
            # chunk-1 v first (its AVs need it from step 6), then
            # chunk-1 ec1 (pair-0 heads 2,3 read it by step 32), then
            # chunk-2/3 units as their x lands
            qk_unit(1, xs1, 0, 0, True, ps, "mm")
            qk_unit(1, xs1, 1, 0, True, ps_sc, "sc")
            pad_fill(2)
            pad_fill(3)
            u2 = list(phase1_units(2, xs2))
            u3 = list(phase1_units(3, xs3))
            by_kind = {k: [u for u in u2 + u3 if u[0] == k]
                       for k in ("qk0", "v", "qk1")}
            v2 = [u for u in by_kind["v"] if u[1].__defaults__[0] == 2]
            v3 = [u for u in by_kind["v"] if u[1].__defaults__[0] == 3]
            p1_rest = deque(
                [("v", lambda i=it: v_unit(1, xs1, i, ps, "mm"))
                 for it in range(4)]
                + [("qk1", lambda q=qk: qk_unit(1, xs1, q, 1, False,
                                                ps, "mm"))
                   for qk in range(2)]
                + by_kind["qk0"][:2] + v2 + by_kind["qk0"][2:] + v3)
            jp1_fill = deque()
            left = phase2(0, p1_rest, 5, proj_sink=jp1_fill,
                          plan={1, 2, 3, 4, 8, 12, 15, 18, 21, 23, 25,
                                27, 29, 31})
            for kind, th in left:
                th()
            # chunk-2/3 ec1 q/k units drain before any proj unit: pair
            # 1's later heads read the tiles they produce
            for item in reversed(by_kind["qk1"]):
                jp1_fill.appendleft(item)
            tail = phase2(1, jp1_fill, 3)
            for i, (kind, th) in enumerate(tail):
                if kind == "proj":
                    th(alt_pool=(i % 2 == 1))
                else:
                    th()

    _patch_nc(nc)
    return nc


_NC_CACHE = None


def _get_nc():
    global _NC_CACHE
    if _NC_CACHE is None:
        _NC_CACHE = build_nc()
    return _NC_CACHE


def make_in_maps(x, w_qkv, w_proj):
    """Shard full inputs into the 8 per-core input maps."""
    scale = np.float32(HD ** -0.5)
    # [t_k, t_q]: valid where t_k <= t_q
    mask01 = np.triu(np.ones((128, 128), dtype=np.float32))
    in_maps = []
    for c in range(N_CORES):
        b, g = divmod(c, TPG)
        rows = slice(EPC * g, EPC * (g + 1))
        xt = np.ascontiguousarray(x[b].T)
        wq = np.ascontiguousarray((w_qkv[rows, :] * scale).T)
        wk = np.ascontiguousarray(w_qkv[D:][rows, :].T)
        wv = np.ascontiguousarray(w_qkv[2 * D:][rows, :].T)
        wp = np.ascontiguousarray(w_proj[:, rows].T)
        in_maps.append({
            "xT": xt, "wqT": wq, "wkT": wk, "wvT": wv, "wpT": wp,
            "mask": mask01,
        })
    return in_maps


def combine_outputs(results, b_proj):
    out = np.empty((B, T, D), dtype=np.float32)
    for b in range(B):
        acc = results[TPG * b]["out_part"].astype(np.float32).copy()
        for g in range(1, TPG):
            acc += results[TPG * b + g]["out_part"]
        out[b] = acc + b_proj[None, :]
    return out


def run(x, w_qkv, w_proj, b_proj, trace=False):
    nc = _get_nc()
    if trace:
        install_ntff_hook()
    in_maps = make_in_maps(np.asarray(x), np.asarray(w_qkv), np.asarray(w_proj))
    res = run_bass_kernel_spmd(nc, in_maps, core_ids=list(range(N_CORES)),
                               trace=trace)
    out = combine_outputs(res.results, np.asarray(b_proj))
    return out, res


def kernel(x, w_qkv, w_proj, b_proj):
    out, _ = run(x, w_qkv, w_proj, b_proj, trace=False)
    return out



# revision 50
# speedup vs baseline: 1.1941x; 1.0807x over previous
"""Multi-head causal attention (B=2, T=2048, D=1024, H=16) on 8 TRN2 NeuronCores.

Sharding: 2-way data parallel over batch x 4-way tensor parallel over heads
(4 heads per core). Each core computes q/k/v projections for its heads,
causal attention, and a partial output projection over its head-dim slice;
the host sums the 4 partials per batch and adds the bias.

Schedule (vs the ~213us v1 baseline; v2 ~188us):
  - All matmuls f32r (full PE rate at moving>=256; narrow score AND
    final AV matmuls are widened to 256 columns to dodge the 4x
    narrow-f32r penalty — the AV gap region of e2 is zeroed on gpsimd).
  - k.T is STACKED: one [2 heads x 64, keys] stationary serves both
    heads of a pair (the zero half-rows of the padded q moving operand
    select the head), halving kT SBUF/pad/copy cost.
  - wq streams as 4 x 256KB pieces at the head of the scalar queue so
    the first projection matmul starts ~10us (x chunk 0 heads the
    sync/gpsimd queues).
  - Phase-1 q/k/v projection blocks borrow the idle score-pool PSUM
    slots for a 4-deep accumulator rotation; PSUM->SBUF copies split
    ACT (idle then) / DVE.
  - Attention runs in two q-chunk-pair stages with a lag-3
    score->exp->AV software pipeline; exp covers both chunks of a pair
    in one wide ACT op; causal-mask multiplies run on GpSimd.
  - Softmax normalize per (ch, jq): DVE reciprocal on the [2,512]
    denominator-row pair, ONE K=2 broadcast matmul through the sel2
    selector stationary, one in-place DVE multiply reading PSUM.  ACT
    does nothing but the exp stream.
  - Projection q-chunks + leftover phase-1 units ride as rationed
    filler inside the attention loops, so their output DMAs overlap
    attention compute.
"""

import sys
import types
from collections import deque

import numpy as np
import orjson

import concourse.bass as bass
import concourse.mybir as mybir
import concourse.tile as tile
from concourse.bass_utils import run_bass_kernel_spmd

# ---------------------------------------------------------------- constants
B, T, D = 2, 2048, 1024
H = 16
HD = D // H  # 64
N_CORES = 8
TPG = 4  # tensor-parallel group size (heads split 4 ways)
HPC = H // TPG  # heads per core = 4
EPC = HPC * HD  # head-dim columns per core = 256
KI = 128  # contraction tile
NT = T // 128  # 16 t-tiles
NQ = T // 512  # 4 q-chunks
DK = D // 128  # 8 d-chunks
N_WARM = 2  # PE warmup matmuls (p-state ramp while the first DMAs land)

F32 = mybir.dt.float32
F32R = mybir.dt.float32r
BF16 = mybir.dt.bfloat16
F16 = mybir.dt.float16


# ------------------------------------------------- walrus single-wait fixup
def _split_excess_waits(bir: bytes) -> bytes:
    """This walrus build accepts at most one sync wait per instruction.
    Hoist excess on_wait entries onto EventSemaphore ops inserted just
    before the offending instruction on the same engine."""
    m = orjson.loads(bir)
    n = 0
    for fn in m["functions"]:
        for bb in fn["blocks"]:
            out = []
            for inst in bb["instructions"]:
                si = inst.get("sync_info")
                waits = (si or {}).get("on_wait") or []
                max_waits = 1
                if len(waits) > max_waits:
                    extra, keep = waits[:-max_waits], waits[-max_waits:]
                    for k in range(len(extra)):
                        out.append({
                            "debug": inst.get("debug", 0),
                            "engine": inst["engine"],
                            "ins": [], "outs": [],
                            "name": f"{inst['name']}-ws{n}-{k}",
                            "opcode": "EventSemaphore",
                            "sync_info": {"on_update": [],
                                          "on_wait": [extra[k]]},
                        })
                    si["on_wait"] = keep
                    n += 1
                out.append(inst)
            bb["instructions"] = out
    return orjson.dumps(m)


def _patch_nc(nc):
    orig = nc.to_json_bytes
    nc.to_json_bytes = lambda: _split_excess_waits(orig())
    return nc


# ------------------------------------------------------ NTFF hook (timing)
def install_ntff_hook():
    """Register the axon NTFF profile hook if the image's antenv lacks it.
    Only needed for trace=True runs (timing); harmless otherwise."""
    try:
        from antenv.axon_hooks import get_axon_ntff_profile_hook  # noqa: F401
        return
    except ImportError:
        pass
    try:
        import antenv
        from trn_agent_boot.trn_boot import _ntff_profile_via_ctypes
    except ImportError:
        return
    mod = types.ModuleType("antenv.axon_hooks")
    mod._hook = _ntff_profile_via_ctypes("/opt/axon/libaxon_pjrt.so")
    mod.set_axon_ntff_profile_hook = lambda h: setattr(mod, "_hook", h)
    mod.get_axon_ntff_profile_hook = lambda: mod._hook
    sys.modules["antenv.axon_hooks"] = mod
    antenv.axon_hooks = mod


# ----------------------------------------------------------- device program
def build_nc():
    nc = bass.Bass(target_bir_lowering=False)

    # DRAM I/O (f32r tensors hold IEEE fp32 bits; numpy sees float32)
    # x and the qkv weights stream in bf16: halves the 8MB x stream
    # that gates the whole phase-1 head (the projections accumulate in
    # f32 PSUM; scores/AV run on the f32-copied q/k/v, so only the
    # projection inputs are quantized)
    xT = nc.dram_tensor("xT", [D, T], F16, kind="ExternalInput")
    wqT = nc.dram_tensor("wqT", [D, EPC], F16, kind="ExternalInput")
    wkT = nc.dram_tensor("wkT", [D, EPC], F16, kind="ExternalInput")
    wvT = nc.dram_tensor("wvT", [D, EPC], F16, kind="ExternalInput")
    wpT = nc.dram_tensor("wpT", [EPC, D], F32R, kind="ExternalInput")
    mask = nc.dram_tensor("mask", [128, 128], F32R, kind="ExternalInput")
    out = nc.dram_tensor("out_part", [T, D], F32, kind="ExternalOutput")

    xTr = xT.rearrange("(ko ki) t -> ki ko t", ki=KI)
    wqTr = wqT.rearrange("(ko ki) e -> ki ko e", ki=KI)
    wkTr = wkT.rearrange("(ko ki) e -> ki ko e", ki=KI)
    wvTr = wvT.rearrange("(ko ki) e -> ki ko e", ki=KI)
    wpTr = wpT.rearrange("(ko ki) e -> ki ko e", ki=KI)

    ACT_COPY = mybir.ActivationFunctionType.Copy
    ACT_EXP = mybir.ActivationFunctionType.Exp

    with tile.TileContext(nc) as tc:
        with (
            tc.tile_pool(name="persist", bufs=1) as persist,
            tc.tile_pool(name="xstream", bufs=2) as xstream,
            tc.tile_pool(name="work", bufs=3) as work,
            tc.tile_pool(name="ps", bufs=2, space="PSUM") as ps,
            tc.tile_pool(name="ps_sc", bufs=2, space="PSUM") as ps_sc,
            tc.tile_pool(name="ps_av", bufs=2, space="PSUM") as ps_av,
            tc.tile_pool(name="outp", bufs=3) as outp,
        ):
            # ---- persistent SBUF state (wq split into per-ko-pair pieces
            # on its own DMA queue: the very first matmul only needs piece
            # 0, so it can start ~2us after the queue opens)
            wq_h = [persist.tile([KI, DK // 2, EPC], F16, name=f"wq{i}")
                    for i in range(2)]
            wk_h = [persist.tile([KI, DK // 2, EPC], F16, name=f"wk{i}")
                    for i in range(2)]
            wv_sb = persist.tile([KI, DK, EPC], F16)
            wp_sb = persist.tile([KI, 2, D], F32R)
            mask_sb = persist.tile([128, 128], F32R)
            # q.T per (head, t-chunk), contraction zero-padded 64 -> 128:
            # f32r matmuls only hit the fast path with a full 128-row
            # stationary.  Head-even tiles hold data in rows 0:64 (zeros
            # below), head-odd in rows 64:128 (zeros above), so one
            # STACKED k stationary [2 heads x 64, keys] serves both heads
            # of a pair: the zero rows of the moving q operand kill the
            # other head's contribution.
            qT_sb = {(h, tch): persist.tile([KI, 512], F32R,
                                            name=f"qT_{h}_{tch}")
                     for h in range(HPC) for tch in range(NQ)}
            # k.T stacked per (head-pair, t-chunk): rows 0:64 head 2ch,
            # rows 64:128 head 2ch+1 (no zero padding needed)
            kT_sb = {(ch, tch): persist.tile([KI, 512], F32R,
                                             name=f"kT_{ch}_{tch}")
                     for ch in range(2) for tch in range(NQ)}
            # v with a ones column for the softmax denominator
            v_sb = [persist.tile([KI, HPC, HD + 1], F32R, name=f"v_{tt}")
                    for tt in range(NT)]
            attnT_sb = {(ch, jq): persist.tile([KI, 512], F32R,
                                               name=f"attnT_{ch}_{jq}")
                        for ch in range(2) for jq in range(NQ)}
            zbias = persist.tile([128, 1], F32)
            ones_f32 = persist.tile([128, HD], F32)
            # denominator-broadcast selectors: ones over partition halves
            ones_u_f = persist.tile([1, 128], F32)
            ones_l_f = persist.tile([1, 128], F32)
            ones_u = persist.tile([1, 128], F32R)
            ones_l = persist.tile([1, 128], F32R)
            zeros_f = persist.tile([HD, 512], F32)
            zf128 = persist.tile([128, 128], F32)
            warm_a = persist.tile([128, 128], BF16)
            warm_b = persist.tile([128, 512], BF16)
            act_warm = persist.tile([1, 1], F32)

            # ---- DMAs. The first q-projection matmul needs only wq
            # piece 0 (256KB, scalar queue head) and xs0[0] (sync queue
            # head), so it can start ~2.5us after the queues open.
            def phase1_dma(tch):
                # per-ko x tiles so the first accumulation matmuls start
                # as soon as their slice lands; each chunk splits across
                # two queues (per-queue DMA bandwidth is ~110-135GB/s)
                xs = [xstream.tile([KI, 512], F16, tag=f"xs{ko}",
                                   name=f"xs_{tch}_{ko}")
                      for ko in range(DK)]
                for ko in range(DK):
                    # interleave the two queues in ko (consumption)
                    # order: the accumulation matmuls eat tiles in ko
                    # order, so alternating queues halves the effective
                    # arrival cadence
                    if tch == 0:
                        xdma = nc.sync if ko % 2 == 0 else nc.gpsimd
                    else:
                        xdma = nc.sync if ko % 2 == 0 else nc.scalar
                    xdma.dma_start(
                        xs[ko][:], xTr[:, ko, tch * 512:(tch + 1) * 512])
                return xs

            # queue plans (per-queue order = arrival order):
            #   scalar: wq p0-p3 (256KB each), wk_hi, x1 ko4-7,
            #           x2 ko4-7, x3 ko4-7
            #   sync:   x0 ko0-3, wk_lo, x1 ko0-3, x2 ko0-3, x3 ko0-3
            #   gpsimd: x0 ko4-7, wv, mask, wp
            for p in range(4):
                nc.scalar.dma_start(wq_h[p // 2][:, 2 * (p % 2):
                                                 2 * (p % 2) + 2, :],
                                    wqTr[:, 2 * p:2 * p + 2, :])
            xs0 = phase1_dma(0)
            nc.sync.dma_start(wk_h[0][:], wkTr[:, 0:4, :])
            nc.scalar.dma_start(wk_h[1][:], wkTr[:, 4:8, :])
            nc.gpsimd.dma_start(wv_sb[:], wvTr[:])
            xs1 = phase1_dma(1)
            nc.gpsimd.dma_start(mask_sb[:], mask[:])
            nc.gpsimd.dma_start(wp_sb[:], wpTr[:])
            xs2 = phase1_dma(2)
            xs3 = phase1_dma(3)

            nc.vector.memset(warm_a[:], 0.0)
            nc.vector.memset(warm_b[:], 0.0)
            # PE p-state warmup: dummy matmuls (no DMA deps) keep the PE
            # busy from t~0 so the clock is ramped when real work arrives.
            for i in range(N_WARM):
                wacc = ps.tile([128, 512], F32, tag="mm", name=f"warm{i}")
                nc.tensor.matmul(wacc[:], warm_a[:], warm_b[:],
                                 start=True, stop=True)
            # ACT warmup: absorb the ~1.3us activation-table load and
            # engine cold start before the first real copy/exp
            nc.scalar.activation(act_warm[:], warm_b[0:1, 0:1],
                                 ACT_EXP, bias=0.0, scale=1.0)

            # zero the complementary q.T half-rows on DVE during the
            # initial DMA wait (chunks 2,3 fill inside phase-1 sections,
            # just ahead of first use).  Stacked k.T needs no padding.
            nc.vector.memset(zeros_f[:], 0.0)
            nc.vector.memset(zf128[:], 0.0)

            def pad_fill(tch):
                for h in range(HPC):
                    rows = slice(HD, 128) if h % 2 == 0 else slice(0, HD)
                    nc.vector.tensor_copy(qT_sb[(h, tch)][rows, :],
                                          zeros_f[:])
            pad_fill(0)
            pad_fill(1)

            nc.vector.memset(zbias[:], 0.0)
            nc.vector.memset(ones_f32[:], 1.0)
            nc.vector.memset(ones_u_f[:], 0.0)
            nc.vector.memset(ones_l_f[:], 0.0)
            nc.vector.tensor_copy(ones_u_f[0:1, 0:HD], ones_f32[0:1, :])
            nc.vector.tensor_copy(ones_l_f[0:1, HD:128], ones_f32[0:1, :])
            nc.vector.tensor_copy(ones_u[:], ones_u_f[:])
            nc.vector.tensor_copy(ones_l[:], ones_l_f[:])
            for tt in range(NT):
                nc.vector.tensor_copy(
                    v_sb[tt][:, :, HD:HD + 1].rearrange("p b c -> p (b c)"),
                    ones_f32[:, 0:HPC])

            # ---- phase 1: q.T/k.T [e,t] and v [t,e] projections per
            # 512-wide t-chunk, decomposed into filler units (one PSUM
            # accumulation group each) so chunks 2,3 can interleave into
            # the ACT-bound attention loops. PSUM->SBUF copies: q/k top
            # half on ACT when it is idle (chunks 0,1), rest on DVE.
            def qk_unit(tch, xs, qk, ec, on_act, acc_pool, acc_tag):
                w_h = wq_h if qk == 0 else wk_h
                acc = acc_pool.tile([128, 512], F32, tag=acc_tag,
                                    name=f"qk_{tch}_{qk}_{ec}")
                for ko in range(DK):
                    nc.tensor.matmul(
                        acc[:],
                        w_h[ko // 4][:, ko % 4, ec * 128:(ec + 1) * 128],
                        xs[ko][:],
                        start=(ko == 0), stop=(ko == DK - 1),
                    )
                if qk == 1:
                    # k stays stacked: one full-height copy
                    if on_act:
                        nc.scalar.activation(
                            kT_sb[(ec, tch)][:], acc[:],
                            ACT_COPY, bias=0.0, scale=1.0)
                    else:
                        nc.vector.tensor_copy(kT_sb[(ec, tch)][:], acc[:])
                    return
                # scatter q heads into their padded tiles at matching
                # row offsets (even: rows 0:64, odd: rows 64:128)
                if on_act:
                    nc.scalar.activation(
                        qT_sb[(2 * ec, tch)][0:HD, :], acc[0:HD, :],
                        ACT_COPY, bias=0.0, scale=1.0)
                else:
                    nc.vector.tensor_copy(qT_sb[(2 * ec, tch)][0:HD, :],
                                          acc[0:HD, :])
                nc.vector.tensor_copy(qT_sb[(2 * ec + 1, tch)][HD:128, :],
                                      acc[HD:128, :])

            def v_unit(tch, xs, it, acc_pool, acc_tag):
                tt = tch * 4 + it
                acc = acc_pool.tile([128, EPC], F32, tag=acc_tag,
                                    name=f"v_{tt}")
                for ko in range(DK):
                    nc.tensor.matmul(
                        acc[:],
                        xs[ko][:, it * 128:(it + 1) * 128],
                        wv_sb[:, ko, :],
                        start=(ko == 0), stop=(ko == DK - 1),
                    )
                for h in range(HPC):
                    nc.vector.tensor_copy(
                        v_sb[tt][:, h, 0:HD], acc[:, h * HD:(h + 1) * HD])

            def pace(n):
                # tiny dummy matmuls into the (phase-1-idle) av pool:
                # keep the PE busy across short DMA waits so the clock
                # p-state ramp (3us continuous -> 2.4GHz) is not reset
                for i in range(n):
                    wacc = ps_av.tile([65, 512], F32, tag="av",
                                      name=f"pace{pace.n}")
                    pace.n += 1
                    nc.tensor.matmul(wacc[:], warm_a[:, 0:65], warm_b[:],
                                     start=True, stop=True)
            pace.n = 0

            def phase1(tch, xs):
                # standalone block (DMA-arrival-bound: extra compute here
                # is free); 4-deep PSUM rotation borrowing the idle score
                # pool's slots.  Unit order matches weight-arrival order
                # (wq pieces first on scalar, wk_h1 on scalar ~21us,
                # wk_h0 on sync ~21us, wv on gpsimd ~25us) and pace
                # matmuls bridge the arrival gaps so the PE p-state ramp
                # is never reset by an idle wait.
                rot = 0
                units = [(0, 0), (0, 1), (1, 1), (1, 0)]
                for i, (qk, ec) in enumerate(units):
                    pool, tag = ((ps, "mm"), (ps_sc, "sc"))[rot % 2]
                    rot += 1
                    qk_unit(tch, xs, qk, ec, True, pool, tag)
                for it in range(4):
                    pool, tag = ((ps, "mm"), (ps_sc, "sc"))[rot % 2]
                    rot += 1
                    v_unit(tch, xs, it, pool, tag)

            def phase1_units(tch, xs):
                """Filler units: ec1 q/k of early chunks ride inside
                attention pair 0; chunk-2/3 units after."""
                for qk in range(2):
                    yield ("qk0", lambda t=tch, q=qk: qk_unit(
                        t, xs, q, 0, False, ps, "mm"))
                for it in range(4):
                    yield ("v", lambda t=tch, i=it: v_unit(
                        t, xs, i, ps, "mm"))
                for qk in range(2):
                    yield ("qk1", lambda t=tch, q=qk: qk_unit(
                        t, xs, q, 1, False, ps, "mm"))

            # ---- phase 2: causal attention for one q-chunk pair,
            # all 4 heads. kt-inner; AV lags the score/exp front by 3
            # steps so it never waits on ACT.
            #
            # Softmax normalize, batched per (ch, jq): attnT holds
            # unnormalized AV; the pair's two denominator rows live in
            # the free-dim halves of one [1,1024] SBUF tile (partition-0
            # only: engines cannot address odd partition starts).  Two
            # accumulating K=1 matmuls broadcast them into one [128,512]
            # PSUM tile, ONE ACT Reciprocal (cost is free-size only)
            # inverts it, and one in-place DVE multiply applies it.
            def normalize_one(jp, ch, jq, drows):
                dr = drows[(ch, jq)]
                bc = ps.tile([128, 512], F32, tag="mm",
                             name=f"bc_{jq}_{ch}")
                nc.tensor.matmul(bc[:], ones_u[:], dr[0:1, 0:512],
                                 start=True, stop=False)
                nc.tensor.matmul(bc[:], ones_l[:], dr[0:1, 512:1024],
                                 start=False, stop=True)
                dl = work.tile([128, 512], F32, tag="dl", bufs=1,
                               name=f"dl_{jq}_{ch}")
                nc.scalar.activation(
                    dl[:], bc[:], mybir.ActivationFunctionType.Ln,
                    bias=zbias[:], scale=1.0)
                rb = work.tile([128, 512], F32, tag="rb", bufs=2,
                               name=f"rb_{jq}_{ch}")
                nc.scalar.activation(rb[:], dl[:], ACT_EXP,
                                     bias=zbias[:], scale=-1.0)
                nc.vector.tensor_mul(attnT_sb[(ch, jq)][:],
                                     attnT_sb[(ch, jq)][:], rb[:])

            # ---- phase 3: partial output projection, one t-tile per
            # unit; ko-outer so both e-chunks reuse the attnT
            # stationary. Out DMAs alternate sync/gpsimd queues (scalar
            # stays free to issue the exp stream).
            def proj_unit(tt, alt_pool=False):
                # tail-drained units alternate into the idle score pool:
                # a 2-slot ring would stall each unit on the previous
                # one's copies, perpetually resetting the PE clock ramp
                pool, tag = (ps_sc, "sc") if alt_pool else (ps, "mm")
                o_sb = outp.tile([128, D], F32, tag="o", name=f"o_{tt}")
                accs = [pool.tile([128, 512], F32, tag=tag,
                                  name=f"p_{tt}_{ec}") for ec in range(2)]
                for ko in range(2):
                    for ec in range(2):
                        nc.tensor.matmul(
                            accs[ec][:],
                            attnT_sb[(ko, tt // 4)][:, (tt % 4) * 128:
                                                    (tt % 4 + 1) * 128],
                            wp_sb[:, ko, ec * 512:(ec + 1) * 512],
                            start=(ko == 0), stop=(ko == 1),
                        )
                r = out[tt * 128:(tt + 1) * 128, :]
                for ec in range(2):
                    # copy then IMMEDIATELY drain that half: the ec0
                    # half's DMA overlaps the ec1 copy
                    nc.vector.tensor_copy(
                        o_sb[:, ec * 512:(ec + 1) * 512], accs[ec][:])
                    if tt >= 12:
                        # jq3 tiles drain after the last exp: 2 queue
                        # pieces per half, scalar joins in
                        qs = ((nc.sync, nc.gpsimd, nc.scalar, nc.sync),
                              (nc.gpsimd, nc.scalar, nc.sync, nc.gpsimd),
                              (nc.scalar, nc.sync, nc.gpsimd, nc.scalar),
                              (nc.sync, nc.gpsimd, nc.scalar, nc.gpsimd),
                              )[tt - 12]
                        for p in range(2):
                            c0_ = ec * 512 + p * 256
                            qs[2 * ec + p].dma_start(
                                r[:, c0_:c0_ + 256],
                                o_sb[:, c0_:c0_ + 256])
                    elif tt >= 8:
                        e1, e2_ = ((nc.sync, nc.gpsimd),
                                   (nc.gpsimd, nc.sync))[tt % 2]
                        (e1 if ec == 0 else e2_).dma_start(
                            r[:, ec * 512:(ec + 1) * 512],
                            o_sb[:, ec * 512:(ec + 1) * 512])
                    elif ec == 1:
                        dma_eng = nc.sync if tt % 2 == 0 else nc.gpsimd
                        dma_eng.dma_start(r[:], o_sb[:])

            def phase2(jp, fillers, stride, proj_sink=None, plan=None):
                """Attention for q-chunk pair jp. Between kt steps,
                drain one filler unit every `stride` steps (the loop is
                ACT-bound; fillers soak up idle PE cycles). When a jq's
                normalize completes, its projection units join
                `proj_sink` (default: this pair's own filler queue).
                Undrained fillers are returned to the caller."""
                pair = (2 * jp, 2 * jp + 1)
                if proj_sink is None:
                    proj_sink = fillers
                drows = {}
                nstep = 0
                cooldown = [0]

                def emit_av(h, kt, avs, exps):
                    for jq in sorted(exps):
                        rel0 = kt - 4 * jq
                        c0 = 128 * max(rel0, 0)
                        # the 128-col final AV runs at 1/4 rate (f32r
                        # narrow); widen to 256 (the 256:384 gap of e2
                        # was zeroed on DVE alongside the mask mult)
                        c0m = min(c0, 256)
                        nc.tensor.matmul(
                            avs[jq][:, c0m:],
                            v_sb[kt][:, h, :],
                            exps[jq][:, c0m:],
                            start=(kt == 0), stop=(kt == 4 * jq + 3),
                        )
                    for jq in sorted(exps):
                        if kt == 4 * jq + 3:
                            p0 = (h % 2) * HD
                            av = avs[jq]
                            nc.vector.tensor_copy(
                                attnT_sb[(h // 2, jq)][p0:p0 + HD, :],
                                av[0:HD, :])
                            if h % 2 == 0:
                                dr = work.tile([1, 1024], F32R, tag="dr",
                                               bufs=4, name=f"dr_{h}_{jq}")
                                drows[(h // 2, jq)] = dr
                            dr = drows[(h // 2, jq)]
                            c = (h % 2) * 512
                            nc.vector.tensor_copy(
                                dr[0:1, c:c + 512], av[HD:HD + 1, :])
                            if h % 2 == 1:
                                normalize_one(jp, h // 2, jq, drows)
                                if h == 3:
                                    # both ch normalized: projection of
                                    # this jq becomes filler work
                                    for tt in range(4 * jq, 4 * jq + 4):
                                        proj_sink.append(
                                            ("proj",
                                             lambda t=tt, **kw:
                                             proj_unit(t, **kw)))

                for h in range(HPC):
                    avs = {jq: ps_av.tile([HD + 1, 512], F32, tag="av",
                                          name=f"av_{h}_{jq}")
                           for jq in pair}
                    pipeq = []
                    for kt in range(4 * (pair[1] + 1)):
                        jqs = [jq for jq in pair if kt < 4 * (jq + 1)]
                        # both q-chunks' scores into one 2-bank psum tile
                        # so a single wide ACT exp covers them
                        s2 = ps_sc.tile([128, 2, 512], F32, tag="sc",
                                        name=f"s_{h}_{pair[0]}_{kt}")
                        e2 = work.tile([128, 2, 512], F32R, tag="exp",
                                       bufs=4, name=f"e_{h}_{pair[0]}_{kt}")
                        exps = {}
                        c0s = []
                        for i, jq in enumerate(jqs):
                            rel0 = kt - 4 * jq
                            # columns below 128*rel0 are strictly above
                            # the causal diagonal: skipped
                            c0 = 128 * max(rel0, 0)
                            c0s.append(c0)
                            # f32r matmuls with moving dim <256 run at 1/4
                            # rate: widen the score matmul (exp/AV still
                            # use the true c0)
                            c0m = min(c0, 512 - 256)
                            nc.tensor.matmul(
                                s2[:, i, c0m:],
                                kT_sb[(h // 2, kt // 4)][:, (kt % 4) * 128:
                                                         (kt % 4 + 1) * 128],
                                qT_sb[(h, jq)][:, c0m:],
                                start=True, stop=True,
                            )
                            exps[jq] = e2[:, i, :]
                        width = len(jqs) * 512 - c0s[0]
                        sflat = s2.rearrange("p a b -> p (a b)")
                        eflat = e2.rearrange("p a b -> p (a b)")
                        nc.scalar.activation(
                            eflat[:, c0s[0]:c0s[0] + width],
                            sflat[:, c0s[0]:c0s[0] + width],
                            ACT_EXP, bias=zbias[:], scale=1.0)
                        for i, jq in enumerate(jqs):
                            rel0 = kt - 4 * jq
                            if rel0 >= 0:
                                c0 = 128 * rel0
                                nc.gpsimd.tensor_mul(
                                    e2[:, i, c0:c0 + 128],
                                    e2[:, i, c0:c0 + 128],
                                    mask_sb[:])
                            if rel0 == 3:
                                # zero the never-exp'd gap so the AV
                                # matmul can widen to 256 columns
                                nc.vector.tensor_copy(e2[:, i, 256:384],
                                                      zf128[:])
                        pipeq.append((kt, exps))
                        if len(pipeq) > 3:
                            k0, e0 = pipeq.pop(0)
                            emit_av(h, k0, avs, e0)
                        nstep += 1
                        if plan is not None:
                            if fillers and nstep in plan:
                                fillers.popleft()[1]()
                        else:
                            eff = (stride if fillers and
                                   fillers[0][0] != "proj" else stride + 2)
                            if fillers and nstep % eff == 0:
                                fillers.popleft()[1]()
                    for k0, e0 in pipeq:
                        emit_av(h, k0, avs, e0)
                return fillers

            # ---- schedule: x chunks stream in up front; attention pair
            # 0 absorbs phase-1 chunk-2/3 units as PE filler, pair 1
            # absorbs the remaining ec1 q/k units and all output
            # projections of finished q-chunks; only v-units (needed by
            # pair 1's first head) drain as a block between the pairs.
            # ---- schedule. Phase-1 chunks 0,1 run as full blocks (that
            # region is DMA-arrival-bound, PE has slack anyway). The
            # attention loops are only slightly ACT-heavy (~2us headroom
            # in pair 0, ~10us in pair 1), so fillers are rationed.
            phase1(0, xs0)
            # pair 0's first heads only need chunk-1's ec0 q/k tiles;
            # everything else rides as filler on an explicit drain plan:
            # chunk-1 v first (its AVs need it from step 6), then
            # chunk-1 ec1 (pair-0 heads 2,3 read it by step 32), then
            # chunk-2/3 units as their x lands
            qk_unit(1, xs1, 0, 0, True, ps, "mm")
            qk_unit(1, xs1, 1, 0, True, ps_sc, "sc")
            pad_fill(2)
            pad_fill(3)
            u2 = list(phase1_units(2, xs2))
            u3 = list(phase1_units(3, xs3))
            by_kind = {k: [u for u in u2 + u3 if u[0] == k]
                       for k in ("qk0", "v", "qk1")}
            v2 = [u for u in by_kind["v"] if u[1].__defaults__[0] == 2]
            v3 = [u for u in by_kind["v"] if u[1].__defaults__[0] == 3]
            p1_rest = deque(
                [("v", lambda i=it: v_unit(1, xs1, i, ps, "mm"))
                 for it in range(4)]
                + [("qk1", lambda q=qk: qk_unit(1, xs1, q, 1, False,
                                                ps, "mm"))
                   for qk in range(2)]
                + by_kind["qk0"][:2] + v2 + by_kind["qk0"][2:] + v3)
            jp1_fill = deque()
            left = phase2(0, p1_rest, 5, proj_sink=jp1_fill,
                          plan={1, 2, 3, 4, 8, 12, 15, 18, 21, 23, 25,
                                27, 29, 31})
            for kind, th in left:
                th()
            # chunk-2/3 ec1 q/k units drain before any proj unit: pair
            # 1's later heads read the tiles they produce
            for item in reversed(by_kind["qk1"]):
                jp1_fill.appendleft(item)
            tail = phase2(1, jp1_fill, 4)
            for i, (kind, th) in enumerate(tail):
                if kind == "proj":
                    th(alt_pool=(i % 2 == 1))
                else:
                    th()

    _patch_nc(nc)
    return nc


_NC_CACHE = None


def _get_nc():
    global _NC_CACHE
    if _NC_CACHE is None:
        _NC_CACHE = build_nc()
    return _NC_CACHE


def make_in_maps(x, w_qkv, w_proj):
    """Shard full inputs into the 8 per-core input maps."""
    import ml_dtypes
    bf16 = np.float16
    scale = np.float32(HD ** -0.5)
    # [t_k, t_q]: valid where t_k <= t_q
    mask01 = np.triu(np.ones((128, 128), dtype=np.float32))
    in_maps = []
    xtb = [np.ascontiguousarray(x[b].T).astype(bf16) for b in range(B)]
    for c in range(N_CORES):
        b, g = divmod(c, TPG)
        rows = slice(EPC * g, EPC * (g + 1))
        wq = np.ascontiguousarray((w_qkv[rows, :] * scale).T).astype(bf16)
        wk = np.ascontiguousarray(w_qkv[D:][rows, :].T).astype(bf16)
        wv = np.ascontiguousarray(w_qkv[2 * D:][rows, :].T).astype(bf16)
        wp = np.ascontiguousarray(w_proj[:, rows].T)
        in_maps.append({
            "xT": xtb[b], "wqT": wq, "wkT": wk, "wvT": wv, "wpT": wp,
            "mask": mask01,
        })
    return in_maps


def combine_outputs(results, b_proj):
    out = np.empty((B, T, D), dtype=np.float32)
    for b in range(B):
        acc = results[TPG * b]["out_part"].astype(np.float32).copy()
        for g in range(1, TPG):
            acc += results[TPG * b + g]["out_part"]
        out[b] = acc + b_proj[None, :]
    return out


def run(x, w_qkv, w_proj, b_proj, trace=False):
    nc = _get_nc()
    if trace:
        install_ntff_hook()
    in_maps = make_in_maps(np.asarray(x), np.asarray(w_qkv), np.asarray(w_proj))
    res = run_bass_kernel_spmd(nc, in_maps, core_ids=list(range(N_CORES)),
                               trace=trace)
    out = combine_outputs(res.results, np.asarray(b_proj))
    return out, res


def kernel(x, w_qkv, w_proj, b_proj):
    out, _ = run(x, w_qkv, w_proj, b_proj, trace=False)
    return out



# revision 51
# speedup vs baseline: 1.2144x; 1.0170x over previous
"""Multi-head causal attention (B=2, T=2048, D=1024, H=16) on 8 TRN2 NeuronCores.

Sharding: 2-way data parallel over batch x 4-way tensor parallel over heads
(4 heads per core). Each core computes q/k/v projections for its heads,
causal attention, and a partial output projection over its head-dim slice;
the host sums the 4 partials per batch and adds the bias.

Schedule (vs the ~213us v1 baseline; v2 ~188us):
  - All matmuls f32r (full PE rate at moving>=256; narrow score AND
    final AV matmuls are widened to 256 columns to dodge the 4x
    narrow-f32r penalty — the AV gap region of e2 is zeroed on gpsimd).
  - k.T is STACKED: one [2 heads x 64, keys] stationary serves both
    heads of a pair (the zero half-rows of the padded q moving operand
    select the head), halving kT SBUF/pad/copy cost.
  - wq streams as 4 x 256KB pieces at the head of the scalar queue so
    the first projection matmul starts ~10us (x chunk 0 heads the
    sync/gpsimd queues).
  - Phase-1 q/k/v projection blocks borrow the idle score-pool PSUM
    slots for a 4-deep accumulator rotation; PSUM->SBUF copies split
    ACT (idle then) / DVE.
  - Attention runs in two q-chunk-pair stages with a lag-3
    score->exp->AV software pipeline; exp covers both chunks of a pair
    in one wide ACT op; causal-mask multiplies run on GpSimd.
  - Softmax normalize per (ch, jq): DVE reciprocal on the [2,512]
    denominator-row pair, ONE K=2 broadcast matmul through the sel2
    selector stationary, one in-place DVE multiply reading PSUM.  ACT
    does nothing but the exp stream.
  - Projection q-chunks + leftover phase-1 units ride as rationed
    filler inside the attention loops, so their output DMAs overlap
    attention compute.
"""

import sys
import types
from collections import deque

import numpy as np
import orjson

import concourse.bass as bass
import concourse.mybir as mybir
import concourse.tile as tile
from concourse.bass_utils import run_bass_kernel_spmd

# ---------------------------------------------------------------- constants
B, T, D = 2, 2048, 1024
H = 16
HD = D // H  # 64
N_CORES = 8
TPG = 4  # tensor-parallel group size (heads split 4 ways)
HPC = H // TPG  # heads per core = 4
EPC = HPC * HD  # head-dim columns per core = 256
KI = 128  # contraction tile
NT = T // 128  # 16 t-tiles
NQ = T // 512  # 4 q-chunks
DK = D // 128  # 8 d-chunks
N_WARM = 2  # PE warmup matmuls (p-state ramp while the first DMAs land)

F32 = mybir.dt.float32
F32R = mybir.dt.float32r
BF16 = mybir.dt.bfloat16
F16 = mybir.dt.float16


# ------------------------------------------------- walrus single-wait fixup
def _split_excess_waits(bir: bytes) -> bytes:
    """This walrus build accepts at most one sync wait per instruction.
    Hoist excess on_wait entries onto EventSemaphore ops inserted just
    before the offending instruction on the same engine."""
    m = orjson.loads(bir)
    n = 0
    for fn in m["functions"]:
        for bb in fn["blocks"]:
            out = []
            for inst in bb["instructions"]:
                si = inst.get("sync_info")
                waits = (si or {}).get("on_wait") or []
                max_waits = 1
                if len(waits) > max_waits:
                    extra, keep = waits[:-max_waits], waits[-max_waits:]
                    for k in range(len(extra)):
                        out.append({
                            "debug": inst.get("debug", 0),
                            "engine": inst["engine"],
                            "ins": [], "outs": [],
                            "name": f"{inst['name']}-ws{n}-{k}",
                            "opcode": "EventSemaphore",
                            "sync_info": {"on_update": [],
                                          "on_wait": [extra[k]]},
                        })
                    si["on_wait"] = keep
                    n += 1
                out.append(inst)
            bb["instructions"] = out
    return orjson.dumps(m)


def _patch_nc(nc):
    orig = nc.to_json_bytes
    nc.to_json_bytes = lambda: _split_excess_waits(orig())
    return nc


# ------------------------------------------------------ NTFF hook (timing)
def install_ntff_hook():
    """Register the axon NTFF profile hook if the image's antenv lacks it.
    Only needed for trace=True runs (timing); harmless otherwise."""
    try:
        from antenv.axon_hooks import get_axon_ntff_profile_hook  # noqa: F401
        return
    except ImportError:
        pass
    try:
        import antenv
        from trn_agent_boot.trn_boot import _ntff_profile_via_ctypes
    except ImportError:
        return
    mod = types.ModuleType("antenv.axon_hooks")
    mod._hook = _ntff_profile_via_ctypes("/opt/axon/libaxon_pjrt.so")
    mod.set_axon_ntff_profile_hook = lambda h: setattr(mod, "_hook", h)
    mod.get_axon_ntff_profile_hook = lambda: mod._hook
    sys.modules["antenv.axon_hooks"] = mod
    antenv.axon_hooks = mod


# ----------------------------------------------------------- device program
def build_nc():
    nc = bass.Bass(target_bir_lowering=False)

    # DRAM I/O (f32r tensors hold IEEE fp32 bits; numpy sees float32)
    # x and the qkv weights stream in bf16: halves the 8MB x stream
    # that gates the whole phase-1 head (the projections accumulate in
    # f32 PSUM; scores/AV run on the f32-copied q/k/v, so only the
    # projection inputs are quantized)
    xT = nc.dram_tensor("xT", [D, T], F16, kind="ExternalInput")
    wqT = nc.dram_tensor("wqT", [D, EPC], F16, kind="ExternalInput")
    wkT = nc.dram_tensor("wkT", [D, EPC], F16, kind="ExternalInput")
    wvT = nc.dram_tensor("wvT", [D, EPC], F16, kind="ExternalInput")
    wpT = nc.dram_tensor("wpT", [EPC, D], F32R, kind="ExternalInput")
    mask = nc.dram_tensor("mask", [128, 128], F32R, kind="ExternalInput")
    out = nc.dram_tensor("out_part", [T, D], F32, kind="ExternalOutput")

    xTr = xT.rearrange("(ko ki) t -> ki ko t", ki=KI)
    wqTr = wqT.rearrange("(ko ki) e -> ki ko e", ki=KI)
    wkTr = wkT.rearrange("(ko ki) e -> ki ko e", ki=KI)
    wvTr = wvT.rearrange("(ko ki) e -> ki ko e", ki=KI)
    wpTr = wpT.rearrange("(ko ki) e -> ki ko e", ki=KI)

    ACT_COPY = mybir.ActivationFunctionType.Copy
    ACT_EXP = mybir.ActivationFunctionType.Exp

    with tile.TileContext(nc) as tc:
        with (
            tc.tile_pool(name="persist", bufs=1) as persist,
            tc.tile_pool(name="xstream", bufs=2) as xstream,
            tc.tile_pool(name="work", bufs=3) as work,
            tc.tile_pool(name="ps", bufs=2, space="PSUM") as ps,
            tc.tile_pool(name="ps_sc", bufs=2, space="PSUM") as ps_sc,
            tc.tile_pool(name="ps_av", bufs=2, space="PSUM") as ps_av,
            tc.tile_pool(name="outp", bufs=3) as outp,
        ):
            # ---- persistent SBUF state (wq split into per-ko-pair pieces
            # on its own DMA queue: the very first matmul only needs piece
            # 0, so it can start ~2us after the queue opens)
            wq_h = [persist.tile([KI, DK // 2, EPC], F16, name=f"wq{i}")
                    for i in range(2)]
            wk_h = [persist.tile([KI, DK // 2, EPC], F16, name=f"wk{i}")
                    for i in range(2)]
            wv_sb = persist.tile([KI, DK, EPC], F16)
            wp_sb = persist.tile([KI, 2, D], F32R)
            mask_sb = persist.tile([128, 128], F32R)
            # q.T per (head, t-chunk), contraction zero-padded 64 -> 128:
            # f32r matmuls only hit the fast path with a full 128-row
            # stationary.  Head-even tiles hold data in rows 0:64 (zeros
            # below), head-odd in rows 64:128 (zeros above), so one
            # STACKED k stationary [2 heads x 64, keys] serves both heads
            # of a pair: the zero rows of the moving q operand kill the
            # other head's contribution.
            qT_sb = {(h, tch): persist.tile([KI, 512], F32R,
                                            name=f"qT_{h}_{tch}")
                     for h in range(HPC) for tch in range(NQ)}
            # k.T stacked per (head-pair, t-chunk): rows 0:64 head 2ch,
            # rows 64:128 head 2ch+1 (no zero padding needed)
            kT_sb = {(ch, tch): persist.tile([KI, 512], F32R,
                                             name=f"kT_{ch}_{tch}")
                     for ch in range(2) for tch in range(NQ)}
            # v with a ones column for the softmax denominator
            v_sb = [persist.tile([KI, HPC, HD + 1], F32R, name=f"v_{tt}")
                    for tt in range(NT)]
            attnT_sb = {(ch, jq): persist.tile([KI, 512], F32R,
                                               name=f"attnT_{ch}_{jq}")
                        for ch in range(2) for jq in range(NQ)}
            zbias = persist.tile([128, 1], F32)
            ones_f32 = persist.tile([128, HD], F32)
            # denominator-broadcast selectors: ones over partition halves
            ones_u_f = persist.tile([1, 128], F32)
            ones_l_f = persist.tile([1, 128], F32)
            ones_u = persist.tile([1, 128], F32R)
            ones_l = persist.tile([1, 128], F32R)
            zeros_f = persist.tile([HD, 512], F32)
            zf128 = persist.tile([128, 128], F32)
            warm_a = persist.tile([128, 128], BF16)
            warm_b = persist.tile([128, 512], BF16)
            act_warm = persist.tile([1, 1], F32)

            # ---- DMAs. The first q-projection matmul needs only wq
            # piece 0 (256KB, scalar queue head) and xs0[0] (sync queue
            # head), so it can start ~2.5us after the queues open.
            def phase1_dma(tch):
                # per-ko x tiles so the first accumulation matmuls start
                # as soon as their slice lands; each chunk splits across
                # two queues (per-queue DMA bandwidth is ~110-135GB/s)
                xs = [xstream.tile([KI, 512], F16, tag=f"xs{ko}",
                                   name=f"xs_{tch}_{ko}")
                      for ko in range(DK)]
                for ko in range(DK):
                    # interleave the two queues in ko (consumption)
                    # order: the accumulation matmuls eat tiles in ko
                    # order, so alternating queues halves the effective
                    # arrival cadence
                    if tch == 0:
                        xdma = nc.sync if ko % 2 == 0 else nc.gpsimd
                    else:
                        xdma = nc.sync if ko % 2 == 0 else nc.scalar
                    xdma.dma_start(
                        xs[ko][:], xTr[:, ko, tch * 512:(tch + 1) * 512])
                return xs

            # queue plans (per-queue order = arrival order):
            #   scalar: wq p0-p3 (256KB each), wk_hi, x1 ko4-7,
            #           x2 ko4-7, x3 ko4-7
            #   sync:   x0 ko0-3, wk_lo, x1 ko0-3, x2 ko0-3, x3 ko0-3
            #   gpsimd: x0 ko4-7, wv, mask, wp
            for p in range(4):
                nc.scalar.dma_start(wq_h[p // 2][:, 2 * (p % 2):
                                                 2 * (p % 2) + 2, :],
                                    wqTr[:, 2 * p:2 * p + 2, :])
            xs0 = phase1_dma(0)
            nc.sync.dma_start(wk_h[0][:], wkTr[:, 0:4, :])
            nc.scalar.dma_start(wk_h[1][:], wkTr[:, 4:8, :])
            nc.gpsimd.dma_start(wv_sb[:], wvTr[:])
            xs1 = phase1_dma(1)
            nc.gpsimd.dma_start(mask_sb[:], mask[:])
            nc.gpsimd.dma_start(wp_sb[:], wpTr[:])
            xs2 = phase1_dma(2)
            xs3 = phase1_dma(3)

            nc.vector.memset(warm_a[:], 0.0)
            nc.vector.memset(warm_b[:], 0.0)
            # PE p-state warmup: dummy matmuls (no DMA deps) keep the PE
            # busy from t~0 so the clock is ramped when real work arrives.
            for i in range(N_WARM):
                wacc = ps.tile([128, 512], F32, tag="mm", name=f"warm{i}")
                nc.tensor.matmul(wacc[:], warm_a[:], warm_b[:],
                                 start=True, stop=True)
            # ACT warmup: absorb the ~1.3us activation-table load and
            # engine cold start before the first real copy/exp
            nc.scalar.activation(act_warm[:], warm_b[0:1, 0:1],
                                 ACT_EXP, bias=0.0, scale=1.0)

            # zero the complementary q.T half-rows on DVE during the
            # initial DMA wait (chunks 2,3 fill inside phase-1 sections,
            # just ahead of first use).  Stacked k.T needs no padding.
            nc.vector.memset(zeros_f[:], 0.0)
            nc.vector.memset(zf128[:], 0.0)

            def pad_fill(tch):
                for h in range(HPC):
                    rows = slice(HD, 128) if h % 2 == 0 else slice(0, HD)
                    nc.vector.tensor_copy(qT_sb[(h, tch)][rows, :],
                                          zeros_f[:])
            pad_fill(0)
            pad_fill(1)

            nc.vector.memset(zbias[:], 0.0)
            nc.vector.memset(ones_f32[:], 1.0)
            nc.vector.memset(ones_u_f[:], 0.0)
            nc.vector.memset(ones_l_f[:], 0.0)
            nc.vector.tensor_copy(ones_u_f[0:1, 0:HD], ones_f32[0:1, :])
            nc.vector.tensor_copy(ones_l_f[0:1, HD:128], ones_f32[0:1, :])
            nc.vector.tensor_copy(ones_u[:], ones_u_f[:])
            nc.vector.tensor_copy(ones_l[:], ones_l_f[:])
            for tt in range(NT):
                nc.vector.tensor_copy(
                    v_sb[tt][:, :, HD:HD + 1].rearrange("p b c -> p (b c)"),
                    ones_f32[:, 0:HPC])

            # ---- phase 1: q.T/k.T [e,t] and v [t,e] projections per
            # 512-wide t-chunk, decomposed into filler units (one PSUM
            # accumulation group each) so chunks 2,3 can interleave into
            # the ACT-bound attention loops. PSUM->SBUF copies: q/k top
            # half on ACT when it is idle (chunks 0,1), rest on DVE.
            def qk_unit(tch, xs, qk, ec, on_act, acc_pool, acc_tag):
                w_h = wq_h if qk == 0 else wk_h
                acc = acc_pool.tile([128, 512], F32, tag=acc_tag,
                                    name=f"qk_{tch}_{qk}_{ec}")
                for ko in range(DK):
                    nc.tensor.matmul(
                        acc[:],
                        w_h[ko // 4][:, ko % 4, ec * 128:(ec + 1) * 128],
                        xs[ko][:],
                        start=(ko == 0), stop=(ko == DK - 1),
                    )
                if qk == 1:
                    # k stays stacked: one full-height copy
                    if on_act:
                        nc.scalar.activation(
                            kT_sb[(ec, tch)][:], acc[:],
                            ACT_COPY, bias=0.0, scale=1.0)
                    else:
                        nc.vector.tensor_copy(kT_sb[(ec, tch)][:], acc[:])
                    return
                # scatter q heads into their padded tiles at matching
                # row offsets (even: rows 0:64, odd: rows 64:128)
                if on_act:
                    nc.scalar.activation(
                        qT_sb[(2 * ec, tch)][0:HD, :], acc[0:HD, :],
                        ACT_COPY, bias=0.0, scale=1.0)
                else:
                    nc.vector.tensor_copy(qT_sb[(2 * ec, tch)][0:HD, :],
                                          acc[0:HD, :])
                nc.vector.tensor_copy(qT_sb[(2 * ec + 1, tch)][HD:128, :],
                                      acc[HD:128, :])

            def v_unit(tch, xs, it, acc_pool, acc_tag):
                tt = tch * 4 + it
                acc = acc_pool.tile([128, EPC], F32, tag=acc_tag,
                                    name=f"v_{tt}")
                for ko in range(DK):
                    nc.tensor.matmul(
                        acc[:],
                        xs[ko][:, it * 128:(it + 1) * 128],
                        wv_sb[:, ko, :],
                        start=(ko == 0), stop=(ko == DK - 1),
                    )
                for h in range(HPC):
                    nc.vector.tensor_copy(
                        v_sb[tt][:, h, 0:HD], acc[:, h * HD:(h + 1) * HD])

            def pace(n):
                # tiny dummy matmuls into the (phase-1-idle) av pool:
                # keep the PE busy across short DMA waits so the clock
                # p-state ramp (3us continuous -> 2.4GHz) is not reset
                for i in range(n):
                    wacc = ps_av.tile([65, 512], F32, tag="av",
                                      name=f"pace{pace.n}")
                    pace.n += 1
                    nc.tensor.matmul(wacc[:], warm_a[:, 0:65], warm_b[:],
                                     start=True, stop=True)
            pace.n = 0

            def phase1(tch, xs):
                # standalone block (DMA-arrival-bound: extra compute here
                # is free); 4-deep PSUM rotation borrowing the idle score
                # pool's slots.  Unit order matches weight-arrival order
                # (wq pieces first on scalar, wk_h1 on scalar ~21us,
                # wk_h0 on sync ~21us, wv on gpsimd ~25us) and pace
                # matmuls bridge the arrival gaps so the PE p-state ramp
                # is never reset by an idle wait.
                rot = 0
                units = [(0, 0), (0, 1), (1, 1), (1, 0)]
                for i, (qk, ec) in enumerate(units):
                    pool, tag = ((ps, "mm"), (ps_sc, "sc"))[rot % 2]
                    rot += 1
                    qk_unit(tch, xs, qk, ec, True, pool, tag)
                for it in range(4):
                    pool, tag = ((ps, "mm"), (ps_sc, "sc"))[rot % 2]
                    rot += 1
                    v_unit(tch, xs, it, pool, tag)

            def phase1_units(tch, xs):
                """Filler units: ec1 q/k of early chunks ride inside
                attention pair 0; chunk-2/3 units after."""
                for qk in range(2):
                    yield ("qk0", lambda t=tch, q=qk: qk_unit(
                        t, xs, q, 0, False, ps, "mm"))
                for it in range(4):
                    yield ("v", lambda t=tch, i=it: v_unit(
                        t, xs, i, ps, "mm"))
                for qk in range(2):
                    yield ("qk1", lambda t=tch, q=qk: qk_unit(
                        t, xs, q, 1, False, ps, "mm"))

            # ---- phase 2: causal attention for one q-chunk pair,
            # all 4 heads. kt-inner; AV lags the score/exp front by 3
            # steps so it never waits on ACT.
            #
            # Softmax normalize, batched per (ch, jq): attnT holds
            # unnormalized AV; the pair's two denominator rows live in
            # the free-dim halves of one [1,1024] SBUF tile (partition-0
            # only: engines cannot address odd partition starts).  Two
            # accumulating K=1 matmuls broadcast them into one [128,512]
            # PSUM tile, ONE ACT Reciprocal (cost is free-size only)
            # inverts it, and one in-place DVE multiply applies it.
            def normalize_one(jp, ch, jq, drows):
                dr = drows[(ch, jq)]
                bc = ps.tile([128, 512], F32, tag="mm",
                             name=f"bc_{jq}_{ch}")
                nc.tensor.matmul(bc[:], ones_u[:], dr[0:1, 0:512],
                                 start=True, stop=False)
                nc.tensor.matmul(bc[:], ones_l[:], dr[0:1, 512:1024],
                                 start=False, stop=True)
                dl = work.tile([128, 512], F32, tag="dl", bufs=1,
                               name=f"dl_{jq}_{ch}")
                nc.scalar.activation(
                    dl[:], bc[:], mybir.ActivationFunctionType.Ln,
                    bias=zbias[:], scale=1.0)
                rb = work.tile([128, 512], F32, tag="rb", bufs=2,
                               name=f"rb_{jq}_{ch}")
                nc.scalar.activation(rb[:], dl[:], ACT_EXP,
                                     bias=zbias[:], scale=-1.0)
                nc.vector.tensor_mul(attnT_sb[(ch, jq)][:],
                                     attnT_sb[(ch, jq)][:], rb[:])

            # ---- phase 3: partial output projection, one t-tile per
            # unit; ko-outer so both e-chunks reuse the attnT
            # stationary. Out DMAs alternate sync/gpsimd queues (scalar
            # stays free to issue the exp stream).
            def proj_unit(tt, alt_pool=False):
                # tail-drained units alternate into the idle score pool:
                # a 2-slot ring would stall each unit on the previous
                # one's copies, perpetually resetting the PE clock ramp
                pool, tag = (ps_sc, "sc") if alt_pool else (ps, "mm")
                o_sb = outp.tile([128, D], F32, tag="o", name=f"o_{tt}")
                accs = [pool.tile([128, 512], F32, tag=tag,
                                  name=f"p_{tt}_{ec}") for ec in range(2)]
                for ko in range(2):
                    for ec in range(2):
                        nc.tensor.matmul(
                            accs[ec][:],
                            attnT_sb[(ko, tt // 4)][:, (tt % 4) * 128:
                                                    (tt % 4 + 1) * 128],
                            wp_sb[:, ko, ec * 512:(ec + 1) * 512],
                            start=(ko == 0), stop=(ko == 1),
                        )
                r = out[tt * 128:(tt + 1) * 128, :]
                for ec in range(2):
                    # copy then IMMEDIATELY drain that half: the ec0
                    # half's DMA overlaps the ec1 copy
                    nc.vector.tensor_copy(
                        o_sb[:, ec * 512:(ec + 1) * 512], accs[ec][:])
                    if tt >= 12:
                        # jq3 tiles drain after the last exp: 2 queue
                        # pieces per half, scalar joins in
                        qs = ((nc.sync, nc.gpsimd, nc.scalar, nc.sync),
                              (nc.gpsimd, nc.scalar, nc.sync, nc.gpsimd),
                              (nc.scalar, nc.sync, nc.gpsimd, nc.scalar),
                              (nc.sync, nc.gpsimd, nc.scalar, nc.gpsimd),
                              )[tt - 12]
                        for p in range(2):
                            c0_ = ec * 512 + p * 256
                            qs[2 * ec + p].dma_start(
                                r[:, c0_:c0_ + 256],
                                o_sb[:, c0_:c0_ + 256])
                    elif tt >= 8:
                        e1, e2_ = ((nc.sync, nc.gpsimd),
                                   (nc.gpsimd, nc.sync))[tt % 2]
                        (e1 if ec == 0 else e2_).dma_start(
                            r[:, ec * 512:(ec + 1) * 512],
                            o_sb[:, ec * 512:(ec + 1) * 512])
                    elif ec == 1:
                        dma_eng = nc.sync if tt % 2 == 0 else nc.gpsimd
                        dma_eng.dma_start(r[:], o_sb[:])

            def phase2(jp, fillers, stride, proj_sink=None, plan=None):
                """Attention for q-chunk pair jp. Between kt steps,
                drain one filler unit every `stride` steps (the loop is
                ACT-bound; fillers soak up idle PE cycles). When a jq's
                normalize completes, its projection units join
                `proj_sink` (default: this pair's own filler queue).
                Undrained fillers are returned to the caller."""
                pair = (2 * jp, 2 * jp + 1)
                if proj_sink is None:
                    proj_sink = fillers
                drows = {}
                nstep = 0
                cooldown = [0]

                def emit_av(h, kt, avs, exps):
                    for jq in sorted(exps):
                        rel0 = kt - 4 * jq
                        c0 = 128 * max(rel0, 0)
                        # the 128-col final AV runs at 1/4 rate (f32r
                        # narrow); widen to 256 (the 256:384 gap of e2
                        # was zeroed on DVE alongside the mask mult)
                        c0m = min(c0, 256)
                        nc.tensor.matmul(
                            avs[jq][:, c0m:],
                            v_sb[kt][:, h, :],
                            exps[jq][:, c0m:],
                            start=(kt == 0), stop=(kt == 4 * jq + 3),
                        )
                    for jq in sorted(exps):
                        if kt == 4 * jq + 3:
                            p0 = (h % 2) * HD
                            av = avs[jq]
                            nc.vector.tensor_copy(
                                attnT_sb[(h // 2, jq)][p0:p0 + HD, :],
                                av[0:HD, :])
                            if h % 2 == 0:
                                dr = work.tile([1, 1024], F32R, tag="dr",
                                               bufs=4, name=f"dr_{h}_{jq}")
                                drows[(h // 2, jq)] = dr
                            dr = drows[(h // 2, jq)]
                            c = (h % 2) * 512
                            nc.vector.tensor_copy(
                                dr[0:1, c:c + 512], av[HD:HD + 1, :])
                            if h % 2 == 1:
                                normalize_one(jp, h // 2, jq, drows)
                                if h == 3:
                                    # both ch normalized: projection of
                                    # this jq becomes filler work
                                    for tt in range(4 * jq, 4 * jq + 4):
                                        proj_sink.append(
                                            ("proj",
                                             lambda t=tt, **kw:
                                             proj_unit(t, **kw)))

                for h in range(HPC):
                    avs = {jq: ps_av.tile([HD + 1, 512], F32, tag="av",
                                          name=f"av_{h}_{jq}")
                           for jq in pair}
                    pipeq = []
                    for kt in range(4 * (pair[1] + 1)):
                        jqs = [jq for jq in pair if kt < 4 * (jq + 1)]
                        # both q-chunks' scores into one 2-bank psum tile
                        # so a single wide ACT exp covers them
                        s2 = ps_sc.tile([128, 2, 512], F32, tag="sc",
                                        name=f"s_{h}_{pair[0]}_{kt}")
                        e2 = work.tile([128, 2, 512], F32R, tag="exp",
                                       bufs=4, name=f"e_{h}_{pair[0]}_{kt}")
                        exps = {}
                        c0s = []
                        for i, jq in enumerate(jqs):
                            rel0 = kt - 4 * jq
                            # columns below 128*rel0 are strictly above
                            # the causal diagonal: skipped
                            c0 = 128 * max(rel0, 0)
                            c0s.append(c0)
                            # f32r matmuls with moving dim <256 run at 1/4
                            # rate: widen the score matmul (exp/AV still
                            # use the true c0)
                            c0m = min(c0, 512 - 256)
                            nc.tensor.matmul(
                                s2[:, i, c0m:],
                                kT_sb[(h // 2, kt // 4)][:, (kt % 4) * 128:
                                                         (kt % 4 + 1) * 128],
                                qT_sb[(h, jq)][:, c0m:],
                                start=True, stop=True,
                            )
                            exps[jq] = e2[:, i, :]
                        width = len(jqs) * 512 - c0s[0]
                        sflat = s2.rearrange("p a b -> p (a b)")
                        eflat = e2.rearrange("p a b -> p (a b)")
                        nc.scalar.activation(
                            eflat[:, c0s[0]:c0s[0] + width],
                            sflat[:, c0s[0]:c0s[0] + width],
                            ACT_EXP, bias=zbias[:], scale=1.0)
                        for i, jq in enumerate(jqs):
                            rel0 = kt - 4 * jq
                            if rel0 >= 0:
                                c0 = 128 * rel0
                                nc.gpsimd.tensor_mul(
                                    e2[:, i, c0:c0 + 128],
                                    e2[:, i, c0:c0 + 128],
                                    mask_sb[:])
                            if rel0 == 3:
                                # zero the never-exp'd gap so the AV
                                # matmul can widen to 256 columns
                                nc.vector.tensor_copy(e2[:, i, 256:384],
                                                      zf128[:])
                        pipeq.append((kt, exps))
                        if len(pipeq) > 3:
                            k0, e0 = pipeq.pop(0)
                            emit_av(h, k0, avs, e0)
                        nstep += 1
                        if plan is not None:
                            if fillers and nstep in plan:
                                fillers.popleft()[1]()
                        else:
                            eff = (stride if fillers and
                                   fillers[0][0] != "proj" else stride + 1)
                            if fillers and nstep % eff == 0:
                                fillers.popleft()[1]()
                    for k0, e0 in pipeq:
                        emit_av(h, k0, avs, e0)
                return fillers

            # ---- schedule: x chunks stream in up front; attention pair
            # 0 absorbs phase-1 chunk-2/3 units as PE filler, pair 1
            # absorbs the remaining ec1 q/k units and all output
            # projections of finished q-chunks; only v-units (needed by
            # pair 1's first head) drain as a block between the pairs.
            # ---- schedule. Phase-1 chunks 0,1 run as full blocks (that
            # region is DMA-arrival-bound, PE has slack anyway). The
            # attention loops are only slightly ACT-heavy (~2us headroom
            # in pair 0, ~10us in pair 1), so fillers are rationed.
            phase1(0, xs0)
            # pair 0's first heads only need chunk-1's ec0 q/k tiles;
            # everything else rides as filler on an explicit drain plan:
            # chunk-1 v first (its AVs need it from step 6), then
            # chunk-1 ec1 (pair-0 heads 2,3 read it by step 32), then
            # chunk-2/3 units as their x lands
            qk_unit(1, xs1, 0, 0, True, ps, "mm")
            qk_unit(1, xs1, 1, 0, True, ps_sc, "sc")
            pad_fill(2)
            pad_fill(3)
            u2 = list(phase1_units(2, xs2))
            u3 = list(phase1_units(3, xs3))
            by_kind = {k: [u for u in u2 + u3 if u[0] == k]
                       for k in ("qk0", "v", "qk1")}
            v2 = [u for u in by_kind["v"] if u[1].__defaults__[0] == 2]
            v3 = [u for u in by_kind["v"] if u[1].__defaults__[0] == 3]
            p1_rest = deque(
                [("v", lambda i=it: v_unit(1, xs1, i, ps, "mm"))
                 for it in range(4)]
                + [("qk1", lambda q=qk: qk_unit(1, xs1, q, 1, False,
                                                ps, "mm"))
                   for qk in range(2)]
                + by_kind["qk0"][:2] + v2 + by_kind["qk0"][2:] + v3)
            jp1_fill = deque()
            left = phase2(0, p1_rest, 5, proj_sink=jp1_fill,
                          plan={1, 2, 3, 4, 8, 12, 15, 18, 21, 23, 25,
                                27, 29, 31})
            for kind, th in left:
                th()
            # chunk-2/3 ec1 q/k units drain before any proj unit: pair
            # 1's later heads read the tiles they produce
            for item in reversed(by_kind["qk1"]):
                jp1_fill.appendleft(item)
            tail = phase2(1, jp1_fill, 4)
            for i, (kind, th) in enumerate(tail):
                if kind == "proj":
                    th(alt_pool=(i % 2 == 1))
                else:
                    th()

    _patch_nc(nc)
    return nc


_NC_CACHE = None


def _get_nc():
    global _NC_CACHE
    if _NC_CACHE is None:
        _NC_CACHE = build_nc()
    return _NC_CACHE


def make_in_maps(x, w_qkv, w_proj):
    """Shard full inputs into the 8 per-core input maps."""
    import ml_dtypes
    bf16 = np.float16
    scale = np.float32(HD ** -0.5)
    # [t_k, t_q]: valid where t_k <= t_q
    mask01 = np.triu(np.ones((128, 128), dtype=np.float32))
    in_maps = []
    xtb = [np.ascontiguousarray(x[b].T).astype(bf16) for b in range(B)]
    for c in range(N_CORES):
        b, g = divmod(c, TPG)
        rows = slice(EPC * g, EPC * (g + 1))
        wq = np.ascontiguousarray((w_qkv[rows, :] * scale).T).astype(bf16)
        wk = np.ascontiguousarray(w_qkv[D:][rows, :].T).astype(bf16)
        wv = np.ascontiguousarray(w_qkv[2 * D:][rows, :].T).astype(bf16)
        wp = np.ascontiguousarray(w_proj[:, rows].T)
        in_maps.append({
            "xT": xtb[b], "wqT": wq, "wkT": wk, "wvT": wv, "wpT": wp,
            "mask": mask01,
        })
    return in_maps


def combine_outputs(results, b_proj):
    out = np.empty((B, T, D), dtype=np.float32)
    for b in range(B):
        acc = results[TPG * b]["out_part"].astype(np.float32).copy()
        for g in range(1, TPG):
            acc += results[TPG * b + g]["out_part"]
        out[b] = acc + b_proj[None, :]
    return out


def run(x, w_qkv, w_proj, b_proj, trace=False):
    nc = _get_nc()
    if trace:
        install_ntff_hook()
    in_maps = make_in_maps(np.asarray(x), np.asarray(w_qkv), np.asarray(w_proj))
    res = run_bass_kernel_spmd(nc, in_maps, core_ids=list(range(N_CORES)),
                               trace=trace)
    out = combine_outputs(res.results, np.asarray(b_proj))
    return out, res


def kernel(x, w_qkv, w_proj, b_proj):
    out, _ = run(x, w_qkv, w_proj, b_proj, trace=False)
    return out

